# revision 34
# baseline (speedup 1.0000x reference)
"""Bass/Trainium2 kernel for a 6-layer dense transformer LM (BigramLanguageModel).

Sharding (8 cores): core c = (batch b = c//2, seq-half s = c%2).
Each core owns 512 contiguous tokens of one batch: runs the full 6-layer
transformer on its tokens, exchanging per-layer K/V with its pair core via one
combined pairwise AllGather per layer (replica groups [[0,1],[2,3],[4,5],[6,7]]),
then computes logits for its tokens over the FULL vocab (bf16 on the wire).
Output is assembled on the host.

Device-side layout choices:
  - Activations are feature-major [D(6x128 partition chunks), T2=512(free)], so
    every projection uses the natural weight layout as matmul lhsT and produces
    feature-major output with zero transposes anywhere.
  - LayerNorm scales (ln1_s/ln2_s/lnf_s) are folded into the weights on the
    host; LN biases are asserted zero (true for this model family), so the LN
    apply is two DVE ops per chunk with no ScalarE activation. LN statistics
    are accumulated per-chunk immediately after each residual update ("stats
    chasing") so only the short scalar chain remains at the LN point.
  - Attention runs in two phases: phase A processes the core's OWN key half
    straight from SBUF (no collective dependency) while the combined K+V
    AllGather flies; phase B processes the gathered rank-0 half, which is real
    work for seq-half-1 cores and is nulled via a per-core additive exp bias
    (-1e5) on seq-half-0 cores. Causal masking inside phase A needs only one
    [128,128] triangle constant on the diagonal block of each chunk.
  - Scores for a head-pair's two 64-dim halves share one 2-bank PSUM tile so a
    single fused Exp covers both; V is computed token-major so it is directly
    the PV lhsT; a built-in ones-column in V yields the softmax denominator in
    the same matmul. Denominator groups are normalized staggered (after hp3 /
    hp5) so their Ln/Exp chains overlap attention.
  - Softmax skips max-subtraction (|scores*scale| < ~3 for this model family);
    1/x and 1/sqrt(x) are computed as exp(-ln(x)) / exp(-0.5 ln(x)) on the ACT
    engine; every activation used lives in the natural_log_exp_and_others
    table set so exactly one ACT_TABLE_LOAD is emitted.
  - Matmuls in bf16; residual stream fp32; LN statistics via bf16 PE matmuls.
"""

import os
import sys

for _p in ("/opt/trn_rl_repo", "/root/.axon_site/_ro/trn_rl_repo"):
    if os.path.isdir(_p) and _p not in sys.path:
        sys.path.insert(0, _p)

import numpy as np
import ml_dtypes

import concourse.bass as bass
import concourse.mybir as mybir
import concourse.tile as tile
from concourse import bacc
from concourse import bass_utils

F32 = mybir.dt.float32
F32R = mybir.dt.float32r
BF16 = mybir.dt.bfloat16
AF = mybir.ActivationFunctionType
OP = mybir.AluOpType

L = 6
D = 768
H = 12
HD = 64
FF = 3072
V = 32000
VP = 32256  # padded vocab: 63 slices of 512
T = 1024
T2 = 512
B = 4
ND = D // 128   # 6 feature chunks
NF = FF // 128  # 24 ff chunks
NT = T2 // 128  # 4 own-token chunks
NV = VP // 512  # 63 vocab slices
SCALE = HD ** -0.5
EPS = 1e-5

# param columns in the packed per-layer param tile [128, 36]
P_BO, P_B2, P_B1 = 0, 6, 12
NPRM = 36

# combined AllGather payload layout (bf16 elements)
KV_K = ND * 128 * T2          # 393216: K feature-major [ND,128,T2]
KV_V = T2 * 780               # 399360: V token-major [T2, 780]
KV_N = KV_K + KV_V            # per-rank payload

_BUILT = {}


def _build(nlayers=L):
    nc = bacc.Bacc("TRN2", target_bir_lowering=False, debug=False)

    # Pin every activation we use (Exp/Ln/Identity/Relu/Copy) to the single
    # table set that contains them all, so the compiler emits ONE
    # ACT_TABLE_LOAD instead of thrashing sets between LN (ln) and
    # softmax (exp) ~50 times (~2.7us each).
    from concourse.hw_specs import get_activation_tables

    _tabs = get_activation_tables(nc.m.arch)
    _keep = "natural_log_exp_and_others"
    assert _keep in _tabs
    for _fn in (AF.Exp, AF.Ln, AF.Identity, AF.Relu, AF.Copy):
        assert _fn in _tabs[_keep], _fn
    for _n, _s in _tabs.items():
        if _n != _keep:
            for _fn in (AF.Exp, AF.Ln, AF.Identity, AF.Relu, AF.Copy):
                _s.discard(_fn)

    h0_d = nc.dram_tensor("h0", [128, ND, T2], F32, kind="ExternalInput")
    tri_d = nc.dram_tensor("tri", [128, 128], BF16, kind="ExternalInput")
    bB_d = nc.dram_tensor("biasB", [128, 1], F32, kind="ExternalInput")
    wq_d = nc.dram_tensor("wq_t", [nlayers, ND, 128, ND, 128], BF16, kind="ExternalInput")
    wk_d = nc.dram_tensor("wk_t", [nlayers, ND, 128, ND, 128], BF16, kind="ExternalInput")
    wo_d = nc.dram_tensor("wo_t", [nlayers, ND, 128, ND, 128], BF16, kind="ExternalInput")
    wv_d = nc.dram_tensor("wv_t", [nlayers, 128, ND, D], BF16, kind="ExternalInput")
    w1_d = nc.dram_tensor("w1_t", [nlayers, NF, 128, ND, 128], BF16, kind="ExternalInput")
    w2_d = nc.dram_tensor("w2_t", [nlayers, ND, 128, NF, 128], BF16, kind="ExternalInput")
    wh_d = nc.dram_tensor("wh_t", [NV, 128, ND, 512], BF16, kind="ExternalInput")
    wsk_d = nc.dram_tensor("wsumK", [nlayers, 1, ND, 128], BF16, kind="ExternalInput")
    wsf_d = nc.dram_tensor("wsumF", [nlayers, 1, NF, 128], BF16, kind="ExternalInput")
    prm_d = nc.dram_tensor("prm", [nlayers, 128, NPRM], F32, kind="ExternalInput")
    selA_d = nc.dram_tensor("selA", [6, ND, 128], F32, kind="ExternalInput")
    selB_d = nc.dram_tensor("selB", [6, ND, 128], F32, kind="ExternalInput")
    # logits leave the device as bf16: halves the 66MB output DMA, which
    # otherwise saturates DMA bandwidth and stalls the head GEMM stream.
    out_d = nc.dram_tensor("logits", [T2, VP], BF16, kind="ExternalOutput")

    rg = [[0, 1], [2, 3], [4, 5], [6, 7]]

    with tile.TileContext(nc) as tc:
        with tc.tile_pool(name="pers", bufs=1) as pers, \
             tc.tile_pool(name="sb", bufs=1) as sb, \
             tc.tile_pool(name="w", bufs=1) as wp, \
             tc.tile_pool(name="ps", bufs=1, space="PSUM") as ps, \
             tc.tile_pool(name="dram", bufs=1, space="DRAM") as dram:

            # ---------------- persistent tiles ----------------
            h = [pers.tile([128, T2], F32, name=f"h{m}") for m in range(ND)]
            # bf16 mirror of the residual stream, refreshed by ln_stats after
            # each residual update; consumed as the raw rhs/lhsT of the
            # mean-corrected K and FFN-w1 projections.
            hb = [pers.tile([128, T2], BF16, name=f"hb{m}") for m in range(ND)]
            for m in range(ND):
                nc.sync.dma_start(h[m][:], h0_d[:, m, :])
            tri = pers.tile([128, 128], BF16)
            nc.sync.dma_start(tri[:], tri_d[:])
            bB = pers.tile([128, 1], F32)
            nc.sync.dma_start(bB[:], bB_d[:])

            ones_f = pers.tile([128, 1], F32)
            nc.vector.memset(ones_f[:], 1.0)
            ones_b = pers.tile([128, 1], BF16)     # LN sum lhsT [K=128, M=1]
            nc.vector.tensor_copy(out=ones_b[:], in_=ones_f[:])
            onesM_f = pers.tile([1, 128], F32)
            nc.vector.memset(onesM_f[:], 1.0)
            onesM_r = pers.tile([1, 128], F32R)    # bcast lhsT [K=1, M<=128]
            nc.vector.tensor_copy(out=onesM_r[:], in_=onesM_f[:])
            eps_t = pers.tile([1, 1], F32)
            nc.vector.memset(eps_t[:], EPS)

            selA_r = pers.tile([6, ND, 128], F32R)
            selB_r = pers.tile([6, ND, 128], F32R)
            for _sd, _sr in ((selA_d, selA_r), (selB_d, selB_r)):
                sel_f = sb.tile([6, ND, 128], F32, tag="self", bufs=1, name=f"self_{_sd.name}")
                nc.sync.dma_start(sel_f[:], _sd[:])
                nc.vector.tensor_copy(out=_sr[:], in_=sel_f[:])

            import itertools
            _ln_ctr = itertools.count()

            # ---------------- LN helpers (scale folded into weights) -------
            def ln_stats_tile():
                # s1 and s2 must live in different PSUM banks (start=True
                # clears has_written for the whole bank); "sc" slots are
                # 2-bank score tiles, free outside the attention phase.
                s1 = ps.tile([1, T2], F32, tag="sc", bufs=2, name=f"s1_{next(_ln_ctr)}")
                s2 = ps.tile([1, T2], F32, tag="sc", bufs=2, name=f"s2_{next(_ln_ctr)}")
                return s1, s2

            def ln_stats(s12, src_m, m):
                """Per-chunk stat accumulation right after h[m] updates; also
                refreshes the persistent bf16 mirror hb[m]."""
                s1, s2 = s12
                nc.vector.tensor_copy(out=hb[m][:], in_=src_m[:])
                hsq = sb.tile([128, T2], BF16, tag="hsq", bufs=1)
                nc.vector.tensor_mul(out=hsq[:], in0=hb[m][:], in1=hb[m][:])
                nc.tensor.matmul(s1[:], ones_b[:], hb[m][:], start=(m == 0), stop=(m == ND - 1))
                nc.tensor.matmul(s2[:], ones_b[:], hsq[:], start=(m == 0), stop=(m == ND - 1))

            def ln_chain(s12):
                """Scalar chain: mean, -mean (f32r), rstd broadcast rb_s.
                rstd = exp(-0.5*ln(s2/D - mean^2 + eps))."""
                s1, s2 = s12
                ns = {}
                mean = sb.tile([1, T2], F32, tag="lnstat", bufs=7)
                nc.vector.tensor_scalar_mul(out=mean[:], in0=s1[:], scalar1=1.0 / D)
                nmean_r = sb.tile([1, T2], BF16, tag="lnstat", bufs=7)
                nc.vector.tensor_scalar_mul(out=nmean_r[:], in0=s1[:], scalar1=-1.0 / D)
                msq = sb.tile([1, T2], F32, tag="lnstat", bufs=7)
                nc.vector.tensor_mul(out=msq[:], in0=mean[:], in1=mean[:])
                veps = sb.tile([1, T2], F32, tag="lnstat", bufs=7)
                nc.vector.scalar_tensor_tensor(
                    out=veps[:], in0=s2[:], scalar=1.0 / D, in1=msq[:],
                    op0=OP.mult, op1=OP.subtract)
                lnv = sb.tile([1, T2], F32, tag="lnstat", bufs=7)
                nc.scalar.activation(lnv[:], veps[:], AF.Ln, bias=eps_t[:])
                rstd = sb.tile([1, T2], F32, tag="lnstat", bufs=7)
                nc.scalar.activation(rstd[:], lnv[:], AF.Exp, scale=-0.5)
                rstd_r = sb.tile([1, T2], F32R, tag="lnstat", bufs=7)
                nc.vector.tensor_copy(out=rstd_r[:], in_=rstd[:])
                rb = ps.tile([128, T2], F32, tag="mm", bufs=2)
                nc.tensor.matmul(rb[:], onesM_r[:], rstd_r[:], start=True, stop=True)
                rb_s = sb.tile([128, T2], F32, tag="rb_s", bufs=1)
                nc.vector.tensor_copy(out=rb_s[:], in_=rb[:])
                ns["mean"] = mean
                ns["nmean_r"] = nmean_r
                ns["rstd"] = rstd
                ns["rstd_r"] = rstd_r
                ns["rb_s"] = rb_s
                return ns

            def ln_apply(ns, src, tag="a"):
                """Broadcast mean*rstd + 2-DVE-op apply -> ND bf16 tiles."""
                mr_r = sb.tile([1, T2], F32R, tag="lnstat", bufs=7)
                nc.vector.tensor_mul(out=mr_r[:], in0=ns["mean"][:], in1=ns["rstd"][:])
                a = [sb.tile([128, T2], BF16, tag=f"{tag}{m}", bufs=1,
                             name=f"a_{tag}_{next(_ln_ctr)}_{m}") for m in range(ND)]
                mb = ps.tile([128, T2], F32, tag="mm", bufs=2)
                nc.tensor.matmul(mb[:], onesM_r[:], mr_r[:], start=True, stop=True)
                mb_s = sb.tile([128, T2], F32, tag="mb_s", bufs=1)
                nc.vector.tensor_copy(out=mb_s[:], in_=mb[:])
                for m in range(ND):
                    t1 = sb.tile([128, T2], F32, tag="lnt", bufs=1)
                    nc.vector.scalar_tensor_tensor(
                        out=t1[:], in0=src[m][:], scalar=1.0, in1=ns["rb_s"][:],
                        op0=OP.mult, op1=OP.mult)
                    nc.vector.scalar_tensor_tensor(
                        out=a[m][:], in0=t1[:], scalar=1.0, in1=mb_s[:],
                        op0=OP.mult, op1=OP.subtract)
                return a

            # stats for LN1 of layer 0 (h0 just loaded)
            s12 = ln_stats_tile()
            for m in range(ND):
                ln_stats(s12, h[m], m)

            # ---------------- layers ----------------
            for l in range(nlayers):
                prm = sb.tile([128, NPRM], F32, tag="prm", bufs=2)
                nc.sync.dma_start(prm[:], prm_d[l])
                wsk_r = sb.tile([1, ND, 128], BF16, tag="wskr", bufs=1)
                nc.sync.dma_start(wsk_r[:], wsk_d[l])
                wsf_r = sb.tile([1, NF, 128], BF16, tag="wsfr", bufs=1)
                nc.sync.dma_start(wsf_r[:], wsf_d[l])

                ns1 = ln_chain(s12)

                kv_in = dram.tile([KV_N], BF16, tag="kv_in", bufs=2, name=f"kv_in{l}")
                kv_out = dram.tile([2 * KV_N], BF16, tag="kv_out", bufs=2, name=f"kv_out{l}")
                kin_k = kv_in[0:KV_K].rearrange("(m ki t) -> ki m t", ki=128, t=T2)
                kin_v = kv_in[KV_K:].rearrange("(p f) -> p f", f=780)

                # K projection straight off the raw bf16 residual mirror: the
                # mean term enters as a rank-1 accumulate (wsumK x -mean), the
                # rstd scale rides the PSUM evacuation; nothing waits for the
                # serial LN apply.
                kl = [sb.tile([128, T2], BF16, tag=f"kl{m}", bufs=1, name=f"kl{l}_{m}") for m in range(ND)]
                for m in range(ND):
                    wk_sl = wp.tile([128, ND, 128], BF16, tag="wk", bufs=2)
                    nc.sync.dma_start(wk_sl[:], wk_d[l, m])
                    pk = ps.tile([128, T2], F32, tag="mm", bufs=2)
                    for k in range(ND):
                        nc.tensor.matmul(pk[:], wk_sl[:, k], hb[k][:], start=(k == 0), stop=False)
                    nc.tensor.matmul(pk[:], wsk_r[0:1, m, :], ns1["nmean_r"][:], start=False, stop=True)
                    nc.vector.scalar_tensor_tensor(
                        out=kl[m][:], in0=pk[:], scalar=1.0, in1=ns1["rb_s"][:],
                        op0=OP.mult, op1=OP.mult)
                    nc.sync.dma_start(kin_k[:, m, :], kl[m][:])

                a1 = ln_apply(ns1, h)

                # V projection (token-major, 65-strided heads + ones col)
                vl = [sb.tile([128, 780], BF16, tag=f"vl{t}", bufs=1, name=f"vl{l}_{t}") for t in range(NT)]
                wv_sl = wp.tile([128, ND, D], BF16, tag="wv", bufs=1)
                nc.sync.dma_start(wv_sl[:], wv_d[l])
                for t in range(NT):
                    pv1 = ps.tile([128, T2], F32, tag="mm", bufs=2)
                    pv2 = ps.tile([128, 256], F32, tag="mm", bufs=2)
                    for k in range(ND):
                        lhs = a1[k][:, 128 * t : 128 * t + 128]
                        nc.tensor.matmul(pv1[:], lhs, wv_sl[:, k, 0:512], start=(k == 0), stop=(k == ND - 1))
                        nc.tensor.matmul(pv2[:], lhs, wv_sl[:, k, 512:768], start=(k == 0), stop=(k == ND - 1))
                    vch = vl[t][:].rearrange("p (h e) -> p h e", e=65)
                    nc.vector.tensor_copy(
                        out=vch[:, 0:8, 0:64],
                        in_=pv1[:].rearrange("p (h e) -> p h e", e=64))
                    nc.vector.tensor_copy(
                        out=vch[:, 8:12, 0:64],
                        in_=pv2[:].rearrange("p (h e) -> p h e", e=64))
                    nc.vector.memset(vch[:, :, 64:65], 1.0)
                    nc.sync.dma_start(kin_v[128 * t : 128 * t + 128, :], vl[t][:])

                nc.gpsimd.collective_compute(
                    "AllGather", OP.bypass,
                    ins=[kv_in[:].opt()], outs=[kv_out[:].opt()], replica_groups=rg)

                # Q projection (feature-major, stays local)
                q = [sb.tile([128, T2], BF16, tag=f"q{m}", bufs=1, name=f"q{l}_{m}") for m in range(ND)]
                for m in range(ND):
                    wq_sl = wp.tile([128, ND, 128], BF16, tag="wq", bufs=2)
                    nc.sync.dma_start(wq_sl[:], wq_d[l, m])
                    pq = ps.tile([128, T2], F32, tag="mm", bufs=2)
                    for k in range(ND):
                        nc.tensor.matmul(pq[:], wq_sl[:, k], a1[k][:], start=(k == 0), stop=(k == ND - 1))
                    nc.vector.tensor_copy(out=q[m][:], in_=pq[:])

                # gathered rank-0 half: K (feature-major) / V-hat (token-major)
                kg = sb.tile([128, ND, T2], BF16, tag="kg", bufs=1)
                nc.sync.dma_start(
                    kg[:], kv_out[0:KV_K].rearrange("(m ki t) -> ki m t", ki=128, t=T2))
                vg = sb.tile([128, NT, 780], BF16, tag="vg", bufs=1)
                nc.sync.dma_start(
                    vg[:], kv_out[KV_K : KV_K + KV_V].rearrange("(to ti f) -> ti to f", ti=128, f=780))

                # ---- attention ----
                # phase A: own keys from SBUF (kl/vl), causal-trimmed, triangle
                # mask on the diagonal 128-query block only.
                # phase B: gathered rank-0 keys, full 512 queries; contribution
                # nulled on seq-half-0 cores via additive exp bias.
                o = [sb.tile([128, T2], BF16, tag=f"o{m}", bufs=1, name=f"o{l}_{m}") for m in range(ND)]
                poA_s = [sb.tile([65, 2, T2], BF16, tag=f"poa{m}", bufs=1, name=f"poa{l}_{m}") for m in range(ND)]
                dng = [sb.tile([6, T2], F32, tag=f"dn{g}", bufs=1, name=f"dn{l}_{g}") for g in range(2)]

                for hp in range(ND):
                    poA = ps.tile([65, 2, T2], F32, tag="po", bufs=1)
                    for tk in range(NT):
                        qlo = 128 * tk
                        s2b = ps.tile([128, 2, T2], F32, tag="sc", bufs=2)
                        for j in range(2):
                            nc.tensor.matmul(
                                s2b[:, j, qlo:],
                                kl[hp][64 * j : 64 * j + 64, qlo : qlo + 128],
                                q[hp][64 * j : 64 * j + 64, qlo:],
                                start=True, stop=True)
                        p2 = sb.tile([128, 2, T2], BF16, tag="p", bufs=2)
                        nc.scalar.activation(p2[:, :, qlo:], s2b[:, :, qlo:], AF.Exp, scale=SCALE)
                        for j in range(2):
                            nc.vector.tensor_mul(
                                out=p2[:, j, qlo : qlo + 128],
                                in0=p2[:, j, qlo : qlo + 128], in1=tri[:])
                            nc.tensor.matmul(
                                poA[:, j, qlo:],
                                vl[tk][:, 65 * (2 * hp + j) : 65 * (2 * hp + j) + 65],
                                p2[:, j, qlo:],
                                start=(tk == 0), stop=(tk == NT - 1))
                    nc.vector.tensor_copy(out=poA_s[hp][:], in_=poA[:])

                for hp in range(ND):
                    poB = ps.tile([65, 2, T2], F32, tag="po", bufs=1)
                    for tr in range(NT):
                        s2b = ps.tile([128, 2, T2], F32, tag="sc", bufs=2)
                        for j in range(2):
                            nc.tensor.matmul(
                                s2b[:, j, :],
                                kg[64 * j : 64 * j + 64, hp, 128 * tr : 128 * tr + 128],
                                q[hp][64 * j : 64 * j + 64, :],
                                start=True, stop=True)
                        p2 = sb.tile([128, 2, T2], BF16, tag="p", bufs=2)
                        nc.scalar.activation(p2[:], s2b[:], AF.Exp, scale=SCALE, bias=bB[:, 0:1])
                        for j in range(2):
                            nc.tensor.matmul(
                                poB[:, j, :],
                                vg[:, tr, 65 * (2 * hp + j) : 65 * (2 * hp + j) + 65],
                                p2[:, j, :],
                                start=(tr == 0), stop=(tr == NT - 1))
                    # combine phases; split the ones-row into the denominator tile
                    g = hp // 3
                    for j in range(2):
                        hi = 2 * hp + j
                        nc.vector.tensor_tensor(
                            out=o[hp][64 * j : 64 * j + 64, :],
                            in0=poB[0:64, j, :], in1=poA_s[hp][0:64, j, :], op=OP.add)
                        dtmp = sb.tile([1, T2], F32, tag="dtmp", bufs=1)
                        nc.vector.tensor_tensor(
                            out=dtmp[:], in0=poB[64:65, j, :], in1=poA_s[hp][64:65, j, :], op=OP.add)
                        nc.sync.dma_start(dng[g][(hi - 6 * g) : (hi - 6 * g) + 1, :], dtmp[:])
                    if hp in (3, 5):
                        # normalize group ng (0 after hp3 so its Ln/Exp chain
                        # ran during hp3's attention; 1 at the end):
                        # 1/denom = exp(-ln(denom)); head broadcast via selector
                        ng = 0 if hp == 3 else 1
                        sel = selA_r if ng == 0 else selB_r
                        nc.scalar.activation(dng[ng][:], dng[ng][:], AF.Ln)
                        rec_r = sb.tile([6, T2], F32R, tag=f"recr{ng}", bufs=1, name=f"recr{l}_{ng}")
                        nc.scalar.activation(rec_r[:], dng[ng][:], AF.Exp, scale=-1.0)
                        for m in range(3 * ng, 3 * ng + 3):
                            dnb = ps.tile([128, T2], F32, tag="mm", bufs=2)
                            nc.tensor.matmul(dnb[:], sel[:, m, :], rec_r[:], start=True, stop=True)
                            nc.vector.scalar_tensor_tensor(
                                out=o[m][:], in0=o[m][:], scalar=1.0,
                                in1=dnb[:], op0=OP.mult, op1=OP.mult)

                # output projection + residual; LN2 stats chase the h updates
                s12 = ln_stats_tile()
                for m in range(ND):
                    wo_sl = wp.tile([128, ND, 128], BF16, tag="wo", bufs=2)
                    nc.sync.dma_start(wo_sl[:], wo_d[l, m])
                    pw = ps.tile([128, T2], F32, tag="mm", bufs=2)
                    for k in range(ND):
                        nc.tensor.matmul(pw[:], wo_sl[:, k], o[k][:], start=(k == 0), stop=(k == ND - 1))
                    tt = sb.tile([128, T2], F32, tag="res", bufs=2)
                    nc.scalar.activation(tt[:], pw[:], AF.Identity, bias=prm[:, P_BO + m : P_BO + m + 1])
                    nc.vector.tensor_tensor(out=h[m][:], in0=h[m][:], in1=tt[:], op=OP.add)
                    ln_stats(s12, h[m], m)

                # FFN, raw-path: w1 projects the raw mirror with a rank-1 mean
                # correction; relu commutes with the (positive) per-token rstd,
                # which is deferred to the w2 evacuation (needs b1 == b2 == 0).
                ns2 = ln_chain(s12)
                f = [sb.tile([128, T2], BF16, tag=f"f{fc}", bufs=1, name=f"f{l}_{fc}") for fc in range(NF)]
                for fc in range(NF):
                    w1_sl = wp.tile([128, ND, 128], BF16, tag="w1", bufs=2)
                    nc.sync.dma_start(w1_sl[:], w1_d[l, fc])
                    pf = ps.tile([128, T2], F32, tag=("mm" if fc % 2 == 0 else "sc"), bufs=2)
                    for k in range(ND):
                        nc.tensor.matmul(pf[:], w1_sl[:, k], hb[k][:], start=(k == 0), stop=False)
                    nc.tensor.matmul(pf[:], wsf_r[0:1, fc, :], ns2["nmean_r"][:], start=False, stop=True)
                    nc.scalar.activation(f[fc][:], pf[:], AF.Relu)
                s12 = ln_stats_tile()
                for m in range(ND):
                    w2_sl = wp.tile([128, NF, 128], BF16, tag="w2", bufs=2)
                    nc.sync.dma_start(w2_sl[:], w2_d[l, m])
                    pg = ps.tile([128, T2], F32, tag="mm", bufs=2)
                    for k in range(NF):
                        nc.tensor.matmul(pg[:], w2_sl[:, k], f[k][:], start=(k == 0), stop=(k == NF - 1))
                    tt = sb.tile([128, T2], F32, tag="res", bufs=2)
                    nc.vector.scalar_tensor_tensor(
                        out=tt[:], in0=pg[:], scalar=1.0, in1=ns2["rb_s"][:],
                        op0=OP.mult, op1=OP.mult)
                    nc.vector.tensor_tensor(out=h[m][:], in0=h[m][:], in1=tt[:], op=OP.add)
                    ln_stats(s12, h[m], m)

            # ---------------- final LN + head ----------------
            ns_f = ln_chain(s12)
            hf_t = ln_apply(ns_f, h, tag="a")
            for v in range(NV):
                wh_sl = wp.tile([128, ND, 512], BF16, tag="wh", bufs=2)
                nc.sync.dma_start(wh_sl[:], wh_d[v])
                o_dst = out_d[:, 512 * v : 512 * v + 512].rearrange("(to ti) f -> ti to f", ti=128)
                for t in range(NT):
                    pl = ps.tile([128, 512], F32, tag=("mm" if t % 2 == 0 else "sc"), bufs=2)
                    for k in range(ND):
                        nc.tensor.matmul(
                            pl[:], hf_t[k][:, 128 * t : 128 * t + 128], wh_sl[:, k],
                            start=(k == 0), stop=(k == ND - 1))
                    lg = sb.tile([128, 512], BF16, tag="lg", bufs=2)
                    # evacuate on ScalarE: the DVE is the busier engine here
                    nc.scalar.activation(lg[:], pl[:], AF.Identity)
                    nc.sync.dma_start(o_dst[:, t], lg[:])

    nc.compile()
    if not nc.is_finalized():
        nc.finalize()
    return nc


def _prep_shared(inputs, nlayers):
    bf = ml_dtypes.bfloat16
    for k in ("ln1_b", "ln2_b", "lnf_b", "b1", "b2"):
        assert not np.any(np.asarray(inputs[k])), f"{k} must be zero (folded LN/FFN)"
    wq, wk, wv, wo = (np.asarray(inputs[k], np.float32) for k in ("wq", "wk", "wv", "wo"))
    w1, w2 = np.asarray(inputs["w1"], np.float32), np.asarray(inputs["w2"], np.float32)
    w_head = np.asarray(inputs["w_head"], np.float32)
    ln1_s = np.asarray(inputs["ln1_s"], np.float32)[:nlayers]
    ln2_s = np.asarray(inputs["ln2_s"], np.float32)[:nlayers]
    lnf_s = np.asarray(inputs["lnf_s"], np.float32)

    # fold LN scales into the consuming projections
    wq = wq[:nlayers] * ln1_s[:, :, None]
    wk = wk[:nlayers] * ln1_s[:, :, None]
    wv = wv[:nlayers] * ln1_s[:, :, None]
    w1 = w1[:nlayers] * ln2_s[:, :, None]
    w_head = w_head * lnf_s[:, None]

    def lhst(w, nm, nk):
        # [L, nk*128, nm*128] -> [L, nm, 128, nk, 128] with [l,m,ki,ko,j] = w[l,128ko+ki,128m+j]
        return np.ascontiguousarray(
            w[:nlayers].reshape(nlayers, nk, 128, nm, 128).transpose(0, 3, 2, 1, 4)).astype(bf)

    d = {}
    d["wq_t"] = lhst(wq, ND, ND)
    d["wk_t"] = lhst(wk, ND, ND)
    d["wo_t"] = lhst(wo, ND, ND)
    d["w1_t"] = lhst(w1, NF, ND)
    d["w2_t"] = lhst(w2, ND, NF)
    d["wv_t"] = np.ascontiguousarray(
        wv.reshape(nlayers, ND, 128, D).transpose(0, 2, 1, 3)).astype(bf)
    # column sums of the (scale-folded) K / w1 weights, for the rank-1 mean
    # corrections; summed in fp32 AFTER the bf16 rounding the device will see.
    d["wsumK"] = np.ascontiguousarray(
        wk.astype(bf).astype(np.float32).sum(axis=1).reshape(nlayers, 1, ND, 128)).astype(bf)
    d["wsumF"] = np.ascontiguousarray(
        w1.astype(bf).astype(np.float32).sum(axis=1).reshape(nlayers, 1, NF, 128)).astype(bf)
    whp = np.concatenate([w_head, np.zeros((D, VP - V), np.float32)], axis=1)
    d["wh_t"] = np.ascontiguousarray(
        whp.reshape(ND, 128, NV, 512).transpose(2, 1, 0, 3)).astype(bf)

    prm = np.zeros((nlayers, 128, NPRM), np.float32)

    def chunked(a):  # [L, 768] -> [L, 128, n]
        return np.asarray(a, np.float32)[:nlayers].reshape(nlayers, -1, 128).transpose(0, 2, 1)

    prm[:, :, P_BO : P_BO + ND] = chunked(inputs["bo"])
    prm[:, :, P_B2 : P_B2 + ND] = chunked(inputs["b2"])
    prm[:, :, P_B1 : P_B1 + NF] = chunked(inputs["b1"])
    d["prm"] = np.ascontiguousarray(prm)

    kk = np.arange(128)[:, None]
    qq = np.arange(128)[None, :]
    d["tri"] = np.ascontiguousarray((kk <= qq).astype(bf))

    selA = np.zeros((6, ND, 128), np.float32)
    selB = np.zeros((6, ND, 128), np.float32)
    for hi in range(H):
        tgt = selA if hi < 6 else selB
        tgt[hi % 6, hi // 2, 64 * (hi % 2) : 64 * (hi % 2) + 64] = 1.0
    d["selA"] = selA
    d["selB"] = selB
    return d


_LAST_RESULTS = None


def kernel(x, tok_emb, pos_emb, wq, wk, wv, wo, bo, ln1_s, ln1_b,
           ln2_s, ln2_b, w1, b1, w2, b2, lnf_s, lnf_b, w_head, b_head,
           nlayers=L):
    global _LAST_RESULTS
    if nlayers not in _BUILT:
        _BUILT[nlayers] = _build(nlayers)
    nc = _BUILT[nlayers]

    inputs = dict(x=x, tok_emb=tok_emb, pos_emb=pos_emb, wq=wq, wk=wk, wv=wv,
                  wo=wo, bo=bo, ln1_s=ln1_s, ln1_b=ln1_b, ln2_s=ln2_s,
                  ln2_b=ln2_b, w1=w1, b1=b1, w2=w2, b2=b2, lnf_s=lnf_s,
                  lnf_b=lnf_b, w_head=w_head, b_head=b_head)
    shared = _prep_shared(inputs, nlayers)

    xi = np.asarray(x).astype(np.int64)
    te = np.asarray(tok_emb, np.float32)
    pe = np.asarray(pos_emb, np.float32)[:T]
    h0 = te[xi] + pe[None, :, :]  # [B, T, D] fp32

    in_maps = []
    for c in range(8):
        b, s = c // 2, c % 2
        hc = np.ascontiguousarray(
            h0[b, T2 * s : T2 * (s + 1)].T.reshape(ND, 128, T2).transpose(1, 0, 2))
        m = {"h0": hc,
             "biasB": np.full((128, 1), 0.0 if s == 1 else -1e5, np.float32)}
        m.update(shared)
        in_maps.append(m)

    res = bass_utils.run_bass_kernel_spmd(nc, in_maps, core_ids=list(range(8)))
    _LAST_RESULTS = res

    out = np.empty((B, T, V), np.float32)
    for c in range(8):
        b, s = c // 2, c % 2
        out[b, T2 * s : T2 * (s + 1)] = res.results[c]["logits"][:, :V].astype(np.float32)
    bh = np.asarray(b_head, np.float32)
    if np.any(bh):
        out += bh
    return out


if __name__ == "__main__":
    nl = int(os.environ.get("KERNEL_LAYERS", L))
    _build(nl)
    print("build ok", nl)


# revision 42
# speedup vs baseline: 1.2146x; 1.2146x over previous
"""Bass/Trainium2 kernel for a 6-layer dense transformer LM (BigramLanguageModel).

Sharding (8 cores): core c = (batch b = c//2, seq-half s = c%2).
Each core owns 512 contiguous tokens of one batch: runs the full 6-layer
transformer on its tokens, exchanging per-layer K/V with its pair core via one
combined pairwise AllGather per layer (replica groups [[0,1],[2,3],[4,5],[6,7]]),
then computes logits for its tokens over the FULL vocab (bf16 on the wire).
Output is assembled on the host.

Device-side layout choices:
  - Activations are feature-major [D(6x128 partition chunks), T2=512(free)], so
    every projection uses the natural weight layout as matmul lhsT and produces
    feature-major output with zero transposes anywhere.
  - LayerNorm scales (ln1_s/ln2_s/lnf_s) are folded into the weights on the
    host; LN biases are asserted zero (true for this model family), so the LN
    apply is two DVE ops per chunk with no ScalarE activation. LN statistics
    are accumulated per-chunk immediately after each residual update ("stats
    chasing") so only the short scalar chain remains at the LN point.
  - Attention runs in two phases: phase A processes the core's OWN key half
    straight from SBUF (no collective dependency) while the combined K+V
    AllGather flies; phase B processes the gathered rank-0 half, which is real
    work for seq-half-1 cores and is nulled via a per-core additive exp bias
    (-1e5) on seq-half-0 cores. Causal masking inside phase A needs only one
    [128,128] triangle constant on the diagonal block of each chunk.
  - Scores for a head-pair's two 64-dim halves share one 2-bank PSUM tile so a
    single fused Exp covers both; V is computed token-major so it is directly
    the PV lhsT; a built-in ones-column in V yields the softmax denominator in
    the same matmul. Denominator groups are normalized staggered (after hp3 /
    hp5) so their Ln/Exp chains overlap attention.
  - Softmax skips max-subtraction (|scores*scale| < ~3 for this model family);
    1/x and 1/sqrt(x) are computed as exp(-ln(x)) / exp(-0.5 ln(x)) on the ACT
    engine; every activation used lives in the natural_log_exp_and_others
    table set so exactly one ACT_TABLE_LOAD is emitted.
  - Matmuls in bf16; residual stream fp32; LN statistics via bf16 PE matmuls.
"""

import os
import sys

for _p in ("/opt/trn_rl_repo", "/root/.axon_site/_ro/trn_rl_repo"):
    if os.path.isdir(_p) and _p not in sys.path:
        sys.path.insert(0, _p)

import numpy as np
import ml_dtypes

import concourse.bass as bass
import concourse.mybir as mybir
import concourse.tile as tile
from concourse import bacc
from concourse import bass_utils

F32 = mybir.dt.float32
F32R = mybir.dt.float32r
BF16 = mybir.dt.bfloat16
AF = mybir.ActivationFunctionType
OP = mybir.AluOpType

L = 6
D = 768
H = 12
HD = 64
FF = 3072
V = 32000
VP = 32256  # padded vocab: 63 slices of 512
T = 1024
T2 = 512
B = 4
ND = D // 128   # 6 feature chunks
NF = FF // 128  # 24 ff chunks
NT = T2 // 128  # 4 own-token chunks
NV = VP // 512  # 63 vocab slices
SCALE = HD ** -0.5
EPS = 1e-5

# param columns in the packed per-layer param tile [128, 36]
P_BO, P_B2, P_B1 = 0, 6, 12
NPRM = 36

# combined AllGather payload layout (bf16 elements)
KV_K = ND * 128 * T2          # 393216: K feature-major [ND,128,T2]
KV_V = T2 * 780               # 399360: V token-major [T2, 780]
KV_N = KV_K + KV_V            # per-rank payload

_BUILT = {}


def _build(nlayers=L):
    nc = bacc.Bacc("TRN2", target_bir_lowering=False, debug=False)

    # Pin every activation we use (Exp/Ln/Identity/Relu/Copy) to the single
    # table set that contains them all, so the compiler emits ONE
    # ACT_TABLE_LOAD instead of thrashing sets between LN (ln) and
    # softmax (exp) ~50 times (~2.7us each).
    from concourse.hw_specs import get_activation_tables

    _tabs = get_activation_tables(nc.m.arch)
    _keep = "natural_log_exp_and_others"
    assert _keep in _tabs
    for _fn in (AF.Exp, AF.Ln, AF.Identity, AF.Relu, AF.Copy):
        assert _fn in _tabs[_keep], _fn
    for _n, _s in _tabs.items():
        if _n != _keep:
            for _fn in (AF.Exp, AF.Ln, AF.Identity, AF.Relu, AF.Copy):
                _s.discard(_fn)

    h0_d = nc.dram_tensor("h0", [128, ND, T2], F32, kind="ExternalInput")
    tri_d = nc.dram_tensor("tri", [128, 128], BF16, kind="ExternalInput")
    bB_d = nc.dram_tensor("biasB", [128, 1], F32, kind="ExternalInput")
    wq_d = nc.dram_tensor("wq_t", [nlayers, ND, 128, ND, 128], BF16, kind="ExternalInput")
    wk_d = nc.dram_tensor("wk_t", [nlayers, ND, 128, ND, 128], BF16, kind="ExternalInput")
    wo_d = nc.dram_tensor("wo_t", [nlayers, ND, 128, ND, 128], BF16, kind="ExternalInput")
    wv_d = nc.dram_tensor("wv_t", [nlayers, 128, ND, D], BF16, kind="ExternalInput")
    w1_d = nc.dram_tensor("w1_t", [nlayers, NF, 128, ND, 128], BF16, kind="ExternalInput")
    w2_d = nc.dram_tensor("w2_t", [nlayers, ND, 128, NF, 128], BF16, kind="ExternalInput")
    wh_d = nc.dram_tensor("wh_t", [NV, 128, ND, 512], BF16, kind="ExternalInput")
    wsk_d = nc.dram_tensor("wsumK", [nlayers, 1, ND, 128], BF16, kind="ExternalInput")
    wsf_d = nc.dram_tensor("wsumF", [nlayers, 1, NF, 128], BF16, kind="ExternalInput")
    prm_d = nc.dram_tensor("prm", [nlayers, 128, NPRM], F32, kind="ExternalInput")
    selA_d = nc.dram_tensor("selA", [6, ND, 128], BF16, kind="ExternalInput")
    selB_d = nc.dram_tensor("selB", [6, ND, 128], BF16, kind="ExternalInput")
    # logits leave the device as bf16: halves the 66MB output DMA, which
    # otherwise saturates DMA bandwidth and stalls the head GEMM stream.
    out_d = nc.dram_tensor("logits", [T2, VP], BF16, kind="ExternalOutput")

    rg = [[0, 1], [2, 3], [4, 5], [6, 7]]

    with tile.TileContext(nc) as tc:
        with tc.tile_pool(name="pers", bufs=1) as pers, \
             tc.tile_pool(name="sb", bufs=1) as sb, \
             tc.tile_pool(name="w", bufs=1) as wp, \
             tc.tile_pool(name="ps", bufs=1, space="PSUM") as ps, \
             tc.tile_pool(name="dram", bufs=1, space="DRAM") as dram:

            # ---------------- persistent tiles ----------------
            h = [pers.tile([128, T2], F32, name=f"h{m}") for m in range(ND)]
            # bf16 mirror of the residual stream, refreshed by ln_stats after
            # each residual update; consumed as the raw rhs/lhsT of the
            # mean-corrected K and FFN-w1 projections.
            hb = [pers.tile([128, T2], BF16, name=f"hb{m}") for m in range(ND)]
            for m in range(ND):
                nc.sync.dma_start(h[m][:], h0_d[:, m, :])
            tri = pers.tile([128, 128], BF16)
            nc.sync.dma_start(tri[:], tri_d[:])
            bB = pers.tile([128, 1], F32)
            nc.sync.dma_start(bB[:], bB_d[:])

            ones_f = pers.tile([128, 1], F32)
            nc.vector.memset(ones_f[:], 1.0)
            ones_b = pers.tile([128, 1], BF16)     # LN sum lhsT [K=128, M=1]
            nc.vector.tensor_copy(out=ones_b[:], in_=ones_f[:])
            onesM_f = pers.tile([1, 128], F32)
            nc.vector.memset(onesM_f[:], 1.0)
            onesM_r = pers.tile([1, 128], F32R)    # bcast lhsT [K=1, M<=128]
            nc.vector.tensor_copy(out=onesM_r[:], in_=onesM_f[:])
            eps_t = pers.tile([1, 1], F32)
            nc.vector.memset(eps_t[:], EPS)

            selA_r = pers.tile([6, ND, 128], BF16)
            selB_r = pers.tile([6, ND, 128], BF16)
            nc.sync.dma_start(selA_r[:], selA_d[:])
            nc.sync.dma_start(selB_r[:], selB_d[:])

            import itertools
            _ln_ctr = itertools.count()

            # ---------------- LN helpers (scale folded into weights) -------
            def ln_stats_tile():
                # s1 and s2 must live in different PSUM banks (start=True
                # clears has_written for the whole bank); "sc" slots are
                # 2-bank score tiles, free outside the attention phase.
                s1 = ps.tile([1, T2], F32, tag="sc", bufs=2, name=f"s1_{next(_ln_ctr)}")
                s2 = ps.tile([1, T2], F32, tag="sc", bufs=2, name=f"s2_{next(_ln_ctr)}")
                return s1, s2

            def ln_stats(s12, src_m, m):
                """Per-chunk stat accumulation right after h[m] updates; also
                refreshes the persistent bf16 mirror hb[m]."""
                s1, s2 = s12
                nc.vector.tensor_copy(out=hb[m][:], in_=src_m[:])
                hsq = sb.tile([128, T2], BF16, tag="hsq", bufs=1)
                nc.vector.tensor_mul(out=hsq[:], in0=hb[m][:], in1=hb[m][:])
                nc.tensor.matmul(s1[:], ones_b[:], hb[m][:], start=(m == 0), stop=(m == ND - 1))
                nc.tensor.matmul(s2[:], ones_b[:], hsq[:], start=(m == 0), stop=(m == ND - 1))

            def ln_chain(s12):
                """Scalar chain: mean, -mean (f32r), rstd broadcast rb_s.
                rstd = exp(-0.5*ln(s2/D - mean^2 + eps))."""
                s1, s2 = s12
                ns = {}
                mean = sb.tile([1, T2], F32, tag="lnstat", bufs=7)
                nc.vector.tensor_scalar_mul(out=mean[:], in0=s1[:], scalar1=1.0 / D)
                nmean_r = sb.tile([1, T2], BF16, tag="lnstat", bufs=7)
                nc.vector.tensor_scalar_mul(out=nmean_r[:], in0=s1[:], scalar1=-1.0 / D)
                msq = sb.tile([1, T2], F32, tag="lnstat", bufs=7)
                nc.vector.tensor_mul(out=msq[:], in0=mean[:], in1=mean[:])
                veps = sb.tile([1, T2], F32, tag="lnstat", bufs=7)
                nc.vector.scalar_tensor_tensor(
                    out=veps[:], in0=s2[:], scalar=1.0 / D, in1=msq[:],
                    op0=OP.mult, op1=OP.subtract)
                lnv = sb.tile([1, T2], F32, tag="lnstat", bufs=7)
                nc.scalar.activation(lnv[:], veps[:], AF.Ln, bias=eps_t[:])
                rstd = sb.tile([1, T2], F32, tag="lnstat", bufs=7)
                nc.scalar.activation(rstd[:], lnv[:], AF.Exp, scale=-0.5)
                rstd_r = sb.tile([1, T2], F32R, tag="lnstat", bufs=7)
                nc.vector.tensor_copy(out=rstd_r[:], in_=rstd[:])
                ns["mean"] = mean
                ns["nmean_r"] = nmean_r
                ns["rstd"] = rstd
                ns["rstd_r"] = rstd_r
                return ns

            def ln_chain_bcast(ns):
                """rstd row -> full [128,T2] broadcast. Emitted separately so
                the PE-queue slot lands after enough independent matmuls to
                cover the scalar chain latency (the PE queue is in-order)."""
                rb = ps.tile([128, T2], F32, tag="sc", bufs=2)
                nc.tensor.matmul(rb[:], onesM_r[:], ns["rstd_r"][:], start=True, stop=True)
                rb_s = sb.tile([128, T2], F32, tag="rb_s", bufs=1)
                nc.vector.tensor_copy(out=rb_s[:], in_=rb[:])
                ns["rb_s"] = rb_s

            def ln_apply_start(ns):
                """Broadcast mean*rstd for the 2-DVE-op apply."""
                mr_r = sb.tile([1, T2], F32R, tag="lnstat", bufs=7)
                nc.vector.tensor_mul(out=mr_r[:], in0=ns["mean"][:], in1=ns["rstd"][:])
                mb = ps.tile([128, T2], F32, tag="sc", bufs=2)
                nc.tensor.matmul(mb[:], onesM_r[:], mr_r[:], start=True, stop=True)
                mb_s = sb.tile([128, T2], F32, tag="mb_s", bufs=1)
                nc.vector.tensor_copy(out=mb_s[:], in_=mb[:])
                ns["mb_s"] = mb_s

            def ln_apply_chunk(ns, src_m, a_m):
                t1 = sb.tile([128, T2], F32, tag="lnt", bufs=1)
                nc.vector.scalar_tensor_tensor(
                    out=t1[:], in0=src_m[:], scalar=1.0, in1=ns["rb_s"][:],
                    op0=OP.mult, op1=OP.mult)
                nc.vector.scalar_tensor_tensor(
                    out=a_m[:], in0=t1[:], scalar=1.0, in1=ns["mb_s"][:],
                    op0=OP.mult, op1=OP.subtract)

            def ln_apply(ns, src, tag="a"):
                a = [sb.tile([128, T2], BF16, tag=f"{tag}{m}", bufs=1,
                             name=f"a_{tag}_{next(_ln_ctr)}_{m}") for m in range(ND)]
                if "rb_s" not in ns:
                    ln_chain_bcast(ns)
                ln_apply_start(ns)
                for m in range(ND):
                    ln_apply_chunk(ns, src[m], a[m])
                return a

            # stats for LN1 of layer 0 (h0 just loaded)
            s12 = ln_stats_tile()
            for m in range(ND):
                ln_stats(s12, h[m], m)

            # ---------------- layers ----------------
            for l in range(nlayers):
                prm = sb.tile([128, NPRM], F32, tag="prm", bufs=2)
                nc.sync.dma_start(prm[:], prm_d[l])
                wsk_r = sb.tile([1, ND, 128], BF16, tag="wskr", bufs=1)
                nc.sync.dma_start(wsk_r[:], wsk_d[l])
                wsf_r = sb.tile([1, NF, 128], BF16, tag="wsfr", bufs=1)
                nc.sync.dma_start(wsf_r[:], wsf_d[l])

                ns1 = ln_chain(s12)

                kv_in = dram.tile([KV_N], BF16, tag="kv_in", bufs=2, name=f"kv_in{l}")
                kv_out = dram.tile([2 * KV_N], BF16, tag="kv_out", bufs=2, name=f"kv_out{l}")
                kin_k = kv_in[0:KV_K].rearrange("(m ki t) -> ki m t", ki=128, t=T2)
                kin_v = kv_in[KV_K:].rearrange("(p f) -> p f", f=780)

                # K projection straight off the raw bf16 residual mirror: the
                # mean term enters as a rank-1 accumulate (wsumK x -mean), the
                # rstd scale rides the PSUM evacuation; nothing waits for the
                # serial LN apply. The rank-1 + evacuation of chunk m are
                # emitted one group late (and the rstd broadcast after group 1)
                # so the in-order PE queue never parks on the scalar chain.
                kl = [sb.tile([128, T2], BF16, tag=f"kl{m}", bufs=1, name=f"kl{l}_{m}") for m in range(ND)]
                a1 = [sb.tile([128, T2], BF16, tag=f"a{m}", bufs=1,
                              name=f"a1_{l}_{m}") for m in range(ND)]
                pks = {}

                def k_finish(m):
                    nc.tensor.matmul(pks[m][:], wsk_r[0:1, m, :], ns1["nmean_r"][:], start=False, stop=True)
                    nc.vector.scalar_tensor_tensor(
                        out=kl[m][:], in0=pks[m][:], scalar=1.0, in1=ns1["rb_s"][:],
                        op0=OP.mult, op1=OP.mult)
                    nc.sync.dma_start(kin_k[:, m, :], kl[m][:])

                for m in range(ND):
                    wk_sl = wp.tile([128, ND, 128], BF16, tag="wk", bufs=2)
                    nc.sync.dma_start(wk_sl[:], wk_d[l, m])
                    pk = ps.tile([128, T2], F32, tag="mm", bufs=2)
                    for k in range(ND):
                        nc.tensor.matmul(pk[:], wk_sl[:, k], hb[k][:], start=(k == 0), stop=False)
                    pks[m] = pk
                    if m == 1:
                        ln_chain_bcast(ns1)
                        ln_apply_start(ns1)
                    if m >= 1:
                        k_finish(m - 1)
                    if m >= 2:
                        ln_apply_chunk(ns1, h[m - 2], a1[m - 2])
                k_finish(ND - 1)
                for m in range(ND - 2, ND):
                    ln_apply_chunk(ns1, h[m], a1[m])

                # V projection (token-major, 65-strided heads + ones col)
                vl = [sb.tile([128, 780], BF16, tag=f"vl{t}", bufs=1, name=f"vl{l}_{t}") for t in range(NT)]
                wv_sl = wp.tile([128, ND, D], BF16, tag="wv", bufs=1)
                nc.sync.dma_start(wv_sl[:], wv_d[l])
                for t in range(NT):
                    pv1 = ps.tile([128, T2], F32, tag="mm", bufs=2)
                    pv2 = ps.tile([128, 256], F32, tag="mm", bufs=2)
                    for k in range(ND):
                        lhs = a1[k][:, 128 * t : 128 * t + 128]
                        nc.tensor.matmul(pv1[:], lhs, wv_sl[:, k, 0:512], start=(k == 0), stop=(k == ND - 1))
                        nc.tensor.matmul(pv2[:], lhs, wv_sl[:, k, 512:768], start=(k == 0), stop=(k == ND - 1))
                    vch = vl[t][:].rearrange("p (h e) -> p h e", e=65)
                    nc.vector.tensor_copy(
                        out=vch[:, 0:8, 0:64],
                        in_=pv1[:].rearrange("p (h e) -> p h e", e=64))
                    nc.vector.tensor_copy(
                        out=vch[:, 8:12, 0:64],
                        in_=pv2[:].rearrange("p (h e) -> p h e", e=64))
                    nc.vector.memset(vch[:, :, 64:65], 1.0)
                    nc.sync.dma_start(kin_v[128 * t : 128 * t + 128, :], vl[t][:])

                nc.gpsimd.collective_compute(
                    "AllGather", OP.bypass,
                    ins=[kv_in[:].opt()], outs=[kv_out[:].opt()], replica_groups=rg)

                # Q projection (feature-major, stays local)
                q = [sb.tile([128, T2], BF16, tag=f"q{m}", bufs=1, name=f"q{l}_{m}") for m in range(ND)]
                for m in range(ND):
                    wq_sl = wp.tile([128, ND, 128], BF16, tag="wq", bufs=2)
                    nc.sync.dma_start(wq_sl[:], wq_d[l, m])
                    pq = ps.tile([128, T2], F32, tag="mm", bufs=2)
                    for k in range(ND):
                        nc.tensor.matmul(pq[:], wq_sl[:, k], a1[k][:], start=(k == 0), stop=(k == ND - 1))
                    nc.vector.tensor_copy(out=q[m][:], in_=pq[:])

                # gathered rank-0 half: K (feature-major) / V-hat (token-major)
                kg = sb.tile([128, ND, T2], BF16, tag="kg", bufs=1)
                nc.sync.dma_start(
                    kg[:], kv_out[0:KV_K].rearrange("(m ki t) -> ki m t", ki=128, t=T2))
                vg = sb.tile([128, NT, 780], BF16, tag="vg", bufs=1)
                nc.sync.dma_start(
                    vg[:], kv_out[KV_K : KV_K + KV_V].rearrange("(to ti f) -> ti to f", ti=128, f=780))

                # ---- attention ----
                # phase A: own keys from SBUF (kl/vl), causal-trimmed, triangle
                # mask on the diagonal 128-query block only.
                # phase B: gathered rank-0 keys, full 512 queries; contribution
                # nulled on seq-half-0 cores via additive exp bias.
                o = [sb.tile([128, T2], BF16, tag=f"o{m}", bufs=1, name=f"o{l}_{m}") for m in range(ND)]
                poA_s = [sb.tile([65, 2, T2], BF16, tag=f"poa{m}", bufs=1, name=f"poa{l}_{m}") for m in range(ND)]
                dng = [sb.tile([6, T2], F32, tag=f"dn{g}", bufs=1, name=f"dn{l}_{g}") for g in range(2)]

                for hp in range(ND):
                    poA = ps.tile([65, 2, T2], F32, tag="po", bufs=1)
                    for tk in range(NT):
                        qlo = 128 * tk
                        s2b = ps.tile([128, 2, T2], F32, tag="sc", bufs=2)
                        for j in range(2):
                            nc.tensor.matmul(
                                s2b[:, j, qlo:],
                                kl[hp][64 * j : 64 * j + 64, qlo : qlo + 128],
                                q[hp][64 * j : 64 * j + 64, qlo:],
                                start=True, stop=True)
                        p2 = sb.tile([128, 2, T2], BF16, tag="p", bufs=2)
                        nc.scalar.activation(p2[:, :, qlo:], s2b[:, :, qlo:], AF.Exp, scale=SCALE)
                        for j in range(2):
                            nc.vector.tensor_mul(
                                out=p2[:, j, qlo : qlo + 128],
                                in0=p2[:, j, qlo : qlo + 128], in1=tri[:])
                            nc.tensor.matmul(
                                poA[:, j, qlo:],
                                vl[tk][:, 65 * (2 * hp + j) : 65 * (2 * hp + j) + 65],
                                p2[:, j, qlo:],
                                start=(tk == 0), stop=(tk == NT - 1))
                    nc.vector.tensor_copy(out=poA_s[hp][:], in_=poA[:])

                for hp in range(ND):
                    poB = ps.tile([65, 2, T2], F32, tag="po", bufs=1)
                    for tr in range(NT):
                        s2b = ps.tile([128, 2, T2], F32, tag="sc", bufs=2)
                        for j in range(2):
                            nc.tensor.matmul(
                                s2b[:, j, :],
                                kg[64 * j : 64 * j + 64, hp, 128 * tr : 128 * tr + 128],
                                q[hp][64 * j : 64 * j + 64, :],
                                start=True, stop=True)
                        p2 = sb.tile([128, 2, T2], BF16, tag="p", bufs=2)
                        nc.scalar.activation(p2[:], s2b[:], AF.Exp, scale=SCALE, bias=bB[:, 0:1])
                        for j in range(2):
                            nc.tensor.matmul(
                                poB[:, j, :],
                                vg[:, tr, 65 * (2 * hp + j) : 65 * (2 * hp + j) + 65],
                                p2[:, j, :],
                                start=(tr == 0), stop=(tr == NT - 1))
                    # combine phases; split the ones-row into the denominator tile
                    g = hp // 3
                    for j in range(2):
                        hi = 2 * hp + j
                        nc.vector.tensor_tensor(
                            out=o[hp][64 * j : 64 * j + 64, :],
                            in0=poB[0:64, j, :], in1=poA_s[hp][0:64, j, :], op=OP.add)
                        dtmp = sb.tile([1, T2], F32, tag="dtmp", bufs=2)
                        nc.vector.tensor_tensor(
                            out=dtmp[:], in0=poB[64:65, j, :], in1=poA_s[hp][64:65, j, :], op=OP.add)
                        nc.sync.dma_start(dng[g][(hi - 6 * g) : (hi - 6 * g) + 1, :], dtmp[:])
                    if hp in (3, 5):
                        # normalize group ng (0 after hp3 so its Ln/Exp chain
                        # ran during hp3's attention; 1 at the end):
                        # 1/denom = exp(-ln(denom)); head broadcast via selector
                        ng = 0 if hp == 3 else 1
                        sel = selA_r if ng == 0 else selB_r
                        nc.scalar.activation(dng[ng][:], dng[ng][:], AF.Ln)
                        rec_r = sb.tile([6, T2], BF16, tag=f"recr{ng}", bufs=1, name=f"recr{l}_{ng}")
                        nc.scalar.activation(rec_r[:], dng[ng][:], AF.Exp, scale=-1.0)
                        for m in range(3 * ng, 3 * ng + 3):
                            dnb = ps.tile([128, T2], F32, tag="mm", bufs=2)
                            nc.tensor.matmul(dnb[:], sel[:, m, :], rec_r[:], start=True, stop=True)
                            nc.vector.scalar_tensor_tensor(
                                out=o[m][:], in0=o[m][:], scalar=1.0,
                                in1=dnb[:], op0=OP.mult, op1=OP.mult)

                # output projection + residual; LN2 stats chase the h updates
                s12 = ln_stats_tile()
                for m in range(ND):
                    wo_sl = wp.tile([128, ND, 128], BF16, tag="wo", bufs=2)
                    nc.sync.dma_start(wo_sl[:], wo_d[l, m])
                    pw = ps.tile([128, T2], F32, tag="mm", bufs=2)
                    for k in range(ND):
                        nc.tensor.matmul(pw[:], wo_sl[:, k], o[k][:], start=(k == 0), stop=(k == ND - 1))
                    tt = sb.tile([128, T2], BF16, tag="res", bufs=2)
                    nc.scalar.activation(tt[:], pw[:], AF.Identity, bias=prm[:, P_BO + m : P_BO + m + 1])
                    nc.vector.tensor_tensor(out=h[m][:], in0=h[m][:], in1=tt[:], op=OP.add)
                    ln_stats(s12, h[m], m)

                # FFN, raw-path: w1 projects the raw mirror with a rank-1 mean
                # correction; relu commutes with the (positive) per-token rstd,
                # which is deferred to the w2 evacuation (needs b1 == b2 == 0).
                ns2 = ln_chain(s12)
                f = [sb.tile([128, T2], BF16, tag=f"f{fc}", bufs=1, name=f"f{l}_{fc}") for fc in range(NF)]
                pfs = {}

                def w1_finish(fc):
                    nc.tensor.matmul(pfs[fc][:], wsf_r[0:1, fc, :], ns2["nmean_r"][:], start=False, stop=True)
                    nc.scalar.activation(f[fc][:], pfs[fc][:], AF.Relu)

                for fc in range(NF):
                    w1_sl = wp.tile([128, ND, 128], BF16, tag="w1", bufs=2)
                    nc.sync.dma_start(w1_sl[:], w1_d[l, fc])
                    pf = ps.tile([128, T2], F32, tag=("mm" if fc % 2 == 0 else "sc"), bufs=2)
                    for k in range(ND):
                        nc.tensor.matmul(pf[:], w1_sl[:, k], hb[k][:], start=(k == 0), stop=False)
                    pfs[fc] = pf
                    if fc == 1:
                        ln_chain_bcast(ns2)
                    if fc >= 1:
                        w1_finish(fc - 1)
                w1_finish(NF - 1)
                s12 = ln_stats_tile()
                for m in range(ND):
                    w2_sl = wp.tile([128, NF, 128], BF16, tag="w2", bufs=2)
                    nc.sync.dma_start(w2_sl[:], w2_d[l, m])
                    pg = ps.tile([128, T2], F32, tag="mm", bufs=2)
                    for k in range(NF):
                        nc.tensor.matmul(pg[:], w2_sl[:, k], f[k][:], start=(k == 0), stop=(k == NF - 1))
                    tt = sb.tile([128, T2], BF16, tag="res", bufs=2)
                    nc.vector.scalar_tensor_tensor(
                        out=tt[:], in0=pg[:], scalar=1.0, in1=ns2["rb_s"][:],
                        op0=OP.mult, op1=OP.mult)
                    nc.vector.tensor_tensor(out=h[m][:], in0=h[m][:], in1=tt[:], op=OP.add)
                    ln_stats(s12, h[m], m)

            # ---------------- final LN + head ----------------
            ns_f = ln_chain(s12)
            hf_t = ln_apply(ns_f, h, tag="a")
            for v in range(NV):
                wh_sl = wp.tile([128, ND, 512], BF16, tag="wh", bufs=3)
                nc.sync.dma_start(wh_sl[:], wh_d[v])
                o_dst = out_d[:, 512 * v : 512 * v + 512].rearrange("(to ti) f -> ti to f", ti=128)
                for t in range(NT):
                    pl = ps.tile([128, 512], F32, tag=("mm" if t % 2 == 0 else "sc"), bufs=2)
                    for k in range(ND):
                        nc.tensor.matmul(
                            pl[:], hf_t[k][:, 128 * t : 128 * t + 128], wh_sl[:, k],
                            start=(k == 0), stop=(k == ND - 1))
                    lg = sb.tile([128, 512], BF16, tag="lg", bufs=4)
                    # evacuate on ScalarE: the DVE is the busier engine here
                    nc.scalar.activation(lg[:], pl[:], AF.Identity)
                    nc.sync.dma_start(o_dst[:, t], lg[:])

    nc.compile()
    if not nc.is_finalized():
        nc.finalize()
    return nc


def _prep_shared(inputs, nlayers):
    bf = ml_dtypes.bfloat16
    for k in ("ln1_b", "ln2_b", "lnf_b", "b1", "b2"):
        assert not np.any(np.asarray(inputs[k])), f"{k} must be zero (folded LN/FFN)"
    wq, wk, wv, wo = (np.asarray(inputs[k], np.float32) for k in ("wq", "wk", "wv", "wo"))
    w1, w2 = np.asarray(inputs["w1"], np.float32), np.asarray(inputs["w2"], np.float32)
    w_head = np.asarray(inputs["w_head"], np.float32)
    ln1_s = np.asarray(inputs["ln1_s"], np.float32)[:nlayers]
    ln2_s = np.asarray(inputs["ln2_s"], np.float32)[:nlayers]
    lnf_s = np.asarray(inputs["lnf_s"], np.float32)

    # fold LN scales into the consuming projections
    wq = wq[:nlayers] * ln1_s[:, :, None]
    wk = wk[:nlayers] * ln1_s[:, :, None]
    wv = wv[:nlayers] * ln1_s[:, :, None]
    w1 = w1[:nlayers] * ln2_s[:, :, None]
    w_head = w_head * lnf_s[:, None]

    def lhst(w, nm, nk):
        # [L, nk*128, nm*128] -> [L, nm, 128, nk, 128] with [l,m,ki,ko,j] = w[l,128ko+ki,128m+j]
        return np.ascontiguousarray(
            w[:nlayers].reshape(nlayers, nk, 128, nm, 128).transpose(0, 3, 2, 1, 4)).astype(bf)

    d = {}
    d["wq_t"] = lhst(wq, ND, ND)
    d["wk_t"] = lhst(wk, ND, ND)
    d["wo_t"] = lhst(wo, ND, ND)
    d["w1_t"] = lhst(w1, NF, ND)
    d["w2_t"] = lhst(w2, ND, NF)
    d["wv_t"] = np.ascontiguousarray(
        wv.reshape(nlayers, ND, 128, D).transpose(0, 2, 1, 3)).astype(bf)
    # column sums of the (scale-folded) K / w1 weights, for the rank-1 mean
    # corrections; summed in fp32 AFTER the bf16 rounding the device will see.
    d["wsumK"] = np.ascontiguousarray(
        wk.astype(bf).astype(np.float32).sum(axis=1).reshape(nlayers, 1, ND, 128)).astype(bf)
    d["wsumF"] = np.ascontiguousarray(
        w1.astype(bf).astype(np.float32).sum(axis=1).reshape(nlayers, 1, NF, 128)).astype(bf)
    whp = np.concatenate([w_head, np.zeros((D, VP - V), np.float32)], axis=1)
    d["wh_t"] = np.ascontiguousarray(
        whp.reshape(ND, 128, NV, 512).transpose(2, 1, 0, 3)).astype(bf)

    prm = np.zeros((nlayers, 128, NPRM), np.float32)

    def chunked(a):  # [L, 768] -> [L, 128, n]
        return np.asarray(a, np.float32)[:nlayers].reshape(nlayers, -1, 128).transpose(0, 2, 1)

    prm[:, :, P_BO : P_BO + ND] = chunked(inputs["bo"])
    prm[:, :, P_B2 : P_B2 + ND] = chunked(inputs["b2"])
    prm[:, :, P_B1 : P_B1 + NF] = chunked(inputs["b1"])
    d["prm"] = np.ascontiguousarray(prm)

    kk = np.arange(128)[:, None]
    qq = np.arange(128)[None, :]
    d["tri"] = np.ascontiguousarray((kk <= qq).astype(bf))

    selA = np.zeros((6, ND, 128), np.float32)
    selB = np.zeros((6, ND, 128), np.float32)
    for hi in range(H):
        tgt = selA if hi < 6 else selB
        tgt[hi % 6, hi // 2, 64 * (hi % 2) : 64 * (hi % 2) + 64] = 1.0
    d["selA"] = selA.astype(bf)
    d["selB"] = selB.astype(bf)
    return d


_LAST_RESULTS = None


def kernel(x, tok_emb, pos_emb, wq, wk, wv, wo, bo, ln1_s, ln1_b,
           ln2_s, ln2_b, w1, b1, w2, b2, lnf_s, lnf_b, w_head, b_head,
           nlayers=L):
    global _LAST_RESULTS
    if nlayers not in _BUILT:
        _BUILT[nlayers] = _build(nlayers)
    nc = _BUILT[nlayers]

    inputs = dict(x=x, tok_emb=tok_emb, pos_emb=pos_emb, wq=wq, wk=wk, wv=wv,
                  wo=wo, bo=bo, ln1_s=ln1_s, ln1_b=ln1_b, ln2_s=ln2_s,
                  ln2_b=ln2_b, w1=w1, b1=b1, w2=w2, b2=b2, lnf_s=lnf_s,
                  lnf_b=lnf_b, w_head=w_head, b_head=b_head)
    shared = _prep_shared(inputs, nlayers)

    xi = np.asarray(x).astype(np.int64)
    te = np.asarray(tok_emb, np.float32)
    pe = np.asarray(pos_emb, np.float32)[:T]
    h0 = te[xi] + pe[None, :, :]  # [B, T, D] fp32

    in_maps = []
    for c in range(8):
        b, s = c // 2, c % 2
        hc = np.ascontiguousarray(
            h0[b, T2 * s : T2 * (s + 1)].T.reshape(ND, 128, T2).transpose(1, 0, 2))
        m = {"h0": hc,
             "biasB": np.full((128, 1), 0.0 if s == 1 else -1e5, np.float32)}
        m.update(shared)
        in_maps.append(m)

    res = bass_utils.run_bass_kernel_spmd(nc, in_maps, core_ids=list(range(8)))
    _LAST_RESULTS = res

    out = np.empty((B, T, V), np.float32)
    for c in range(8):
        b, s = c // 2, c % 2
        out[b, T2 * s : T2 * (s + 1)] = res.results[c]["logits"][:, :V].astype(np.float32)
    bh = np.asarray(b_head, np.float32)
    if np.any(bh):
        out += bh
    return out


if __name__ == "__main__":
    nl = int(os.environ.get("KERNEL_LAYERS", L))
    _build(nl)
    print("build ok", nl)


# revision 45
# speedup vs baseline: 1.2189x; 1.0036x over previous
"""Bass/Trainium2 kernel for a 6-layer dense transformer LM (BigramLanguageModel).

Sharding (8 cores): core c = (batch b = c//2, seq-half s = c%2).
Each core owns 512 contiguous tokens of one batch: runs the full 6-layer
transformer on its tokens, exchanging per-layer K/V with its pair core via one
combined pairwise AllGather per layer (replica groups [[0,1],[2,3],[4,5],[6,7]]),
then computes logits for its tokens over the FULL vocab (bf16 on the wire).
Output is assembled on the host.

Device-side layout choices:
  - Activations are feature-major [D(6x128 partition chunks), T2=512(free)], so
    every projection uses the natural weight layout as matmul lhsT and produces
    feature-major output with zero transposes anywhere.
  - LayerNorm scales (ln1_s/ln2_s/lnf_s) are folded into the weights on the
    host; LN biases are asserted zero (true for this model family), so the LN
    apply is two DVE ops per chunk with no ScalarE activation. LN statistics
    are accumulated per-chunk immediately after each residual update ("stats
    chasing") so only the short scalar chain remains at the LN point.
  - Attention runs in two phases: phase A processes the core's OWN key half
    straight from SBUF (no collective dependency) while the combined K+V
    AllGather flies; phase B processes the gathered rank-0 half, which is real
    work for seq-half-1 cores and is nulled via a per-core additive exp bias
    (-1e5) on seq-half-0 cores. Causal masking inside phase A needs only one
    [128,128] triangle constant on the diagonal block of each chunk.
  - Scores for a head-pair's two 64-dim halves share one 2-bank PSUM tile so a
    single fused Exp covers both; V is computed token-major so it is directly
    the PV lhsT; a built-in ones-column in V yields the softmax denominator in
    the same matmul. Denominator groups are normalized staggered (after hp3 /
    hp5) so their Ln/Exp chains overlap attention.
  - Softmax skips max-subtraction (|scores*scale| < ~3 for this model family);
    1/x and 1/sqrt(x) are computed as exp(-ln(x)) / exp(-0.5 ln(x)) on the ACT
    engine; every activation used lives in the natural_log_exp_and_others
    table set so exactly one ACT_TABLE_LOAD is emitted.
  - Matmuls in bf16; residual stream fp32; LN statistics via bf16 PE matmuls.
"""

import os
import sys

for _p in ("/opt/trn_rl_repo", "/root/.axon_site/_ro/trn_rl_repo"):
    if os.path.isdir(_p) and _p not in sys.path:
        sys.path.insert(0, _p)

import numpy as np
import ml_dtypes

import concourse.bass as bass
import concourse.mybir as mybir
import concourse.tile as tile
from concourse import bacc
from concourse import bass_utils

F32 = mybir.dt.float32
F32R = mybir.dt.float32r
BF16 = mybir.dt.bfloat16
AF = mybir.ActivationFunctionType
OP = mybir.AluOpType

L = 6
D = 768
H = 12
HD = 64
FF = 3072
V = 32000
VP = 32256  # padded vocab: 63 slices of 512
T = 1024
T2 = 512
B = 4
ND = D // 128   # 6 feature chunks
NF = FF // 128  # 24 ff chunks
NT = T2 // 128  # 4 own-token chunks
NV = VP // 512  # 63 vocab slices
SCALE = HD ** -0.5
EPS = 1e-5

# param columns in the packed per-layer param tile [128, 36]
P_BO, P_B2, P_B1 = 0, 6, 12
NPRM = 36

# combined AllGather payload layout (bf16 elements)
KV_K = ND * 128 * T2          # 393216: K feature-major [ND,128,T2]
KV_V = T2 * 780               # 399360: V token-major [T2, 780]
KV_N = KV_K + KV_V            # per-rank payload

_BUILT = {}


def _build(nlayers=L):
    nc = bacc.Bacc("TRN2", target_bir_lowering=False, debug=False)

    # Pin every activation we use (Exp/Ln/Identity/Relu/Copy) to the single
    # table set that contains them all, so the compiler emits ONE
    # ACT_TABLE_LOAD instead of thrashing sets between LN (ln) and
    # softmax (exp) ~50 times (~2.7us each).
    from concourse.hw_specs import get_activation_tables

    _tabs = get_activation_tables(nc.m.arch)
    _keep = "natural_log_exp_and_others"
    assert _keep in _tabs
    for _fn in (AF.Exp, AF.Ln, AF.Identity, AF.Relu, AF.Copy):
        assert _fn in _tabs[_keep], _fn
    for _n, _s in _tabs.items():
        if _n != _keep:
            for _fn in (AF.Exp, AF.Ln, AF.Identity, AF.Relu, AF.Copy):
                _s.discard(_fn)

    h0_d = nc.dram_tensor("h0", [128, ND, T2], F32, kind="ExternalInput")
    tri_d = nc.dram_tensor("tri", [128, 128], BF16, kind="ExternalInput")
    bB_d = nc.dram_tensor("biasB", [128, 1], F32, kind="ExternalInput")
    wq_d = nc.dram_tensor("wq_t", [nlayers, ND, 128, ND, 128], BF16, kind="ExternalInput")
    wk_d = nc.dram_tensor("wk_t", [nlayers, ND, 128, ND, 128], BF16, kind="ExternalInput")
    wo_d = nc.dram_tensor("wo_t", [nlayers, ND, 128, ND, 128], BF16, kind="ExternalInput")
    wv_d = nc.dram_tensor("wv_t", [nlayers, 128, ND, D], BF16, kind="ExternalInput")
    w1_d = nc.dram_tensor("w1_t", [nlayers, NF, 128, ND, 128], BF16, kind="ExternalInput")
    w2_d = nc.dram_tensor("w2_t", [nlayers, ND, 128, NF, 128], BF16, kind="ExternalInput")
    wh_d = nc.dram_tensor("wh_t", [NV, 128, ND, 512], BF16, kind="ExternalInput")
    wsk_d = nc.dram_tensor("wsumK", [nlayers, 1, ND, 128], BF16, kind="ExternalInput")
    wsf_d = nc.dram_tensor("wsumF", [nlayers, 1, NF, 128], BF16, kind="ExternalInput")
    prm_d = nc.dram_tensor("prm", [nlayers, 128, NPRM], F32, kind="ExternalInput")
    selA_d = nc.dram_tensor("selA", [6, ND, 128], BF16, kind="ExternalInput")
    selB_d = nc.dram_tensor("selB", [6, ND, 128], BF16, kind="ExternalInput")
    # logits leave the device as bf16: halves the 66MB output DMA, which
    # otherwise saturates DMA bandwidth and stalls the head GEMM stream.
    out_d = nc.dram_tensor("logits", [T2, VP], BF16, kind="ExternalOutput")

    rg = [[0, 1], [2, 3], [4, 5], [6, 7]]

    with tile.TileContext(nc) as tc:
        with tc.tile_pool(name="pers", bufs=1) as pers, \
             tc.tile_pool(name="sb", bufs=1) as sb, \
             tc.tile_pool(name="w", bufs=1) as wp, \
             tc.tile_pool(name="ps", bufs=1, space="PSUM") as ps, \
             tc.tile_pool(name="dram", bufs=1, space="DRAM") as dram:

            # ---------------- persistent tiles ----------------
            h = [pers.tile([128, T2], F32, name=f"h{m}") for m in range(ND)]
            # bf16 mirror of the residual stream, refreshed by ln_stats after
            # each residual update; consumed as the raw rhs/lhsT of the
            # mean-corrected K and FFN-w1 projections.
            hb = [pers.tile([128, T2], BF16, name=f"hb{m}") for m in range(ND)]
            for m in range(ND):
                nc.sync.dma_start(h[m][:], h0_d[:, m, :])
            tri = pers.tile([128, 128], BF16)
            nc.sync.dma_start(tri[:], tri_d[:])
            bB = pers.tile([128, 1], F32)
            nc.sync.dma_start(bB[:], bB_d[:])

            ones_f = pers.tile([128, 1], F32)
            nc.vector.memset(ones_f[:], 1.0)
            ones_b = pers.tile([128, 1], BF16)     # LN sum lhsT [K=128, M=1]
            nc.vector.tensor_copy(out=ones_b[:], in_=ones_f[:])
            onesM_f = pers.tile([1, 128], F32)
            nc.vector.memset(onesM_f[:], 1.0)
            onesM_r = pers.tile([1, 128], F32R)    # bcast lhsT [K=1, M<=128]
            nc.vector.tensor_copy(out=onesM_r[:], in_=onesM_f[:])
            eps_t = pers.tile([1, 1], F32)
            nc.vector.memset(eps_t[:], EPS)

            selA_r = pers.tile([6, ND, 128], BF16)
            selB_r = pers.tile([6, ND, 128], BF16)
            nc.sync.dma_start(selA_r[:], selA_d[:])
            nc.sync.dma_start(selB_r[:], selB_d[:])

            import itertools
            _ln_ctr = itertools.count()

            # ---------------- LN helpers (scale folded into weights) -------
            def ln_stats_tile():
                # s1 and s2 must live in different PSUM banks (start=True
                # clears has_written for the whole bank); "sc" slots are
                # 2-bank score tiles, free outside the attention phase.
                s1 = ps.tile([1, T2], F32, tag="sc", bufs=2, name=f"s1_{next(_ln_ctr)}")
                s2 = ps.tile([1, T2], F32, tag="sc", bufs=2, name=f"s2_{next(_ln_ctr)}")
                return s1, s2

            def ln_stats(s12, src_m, m):
                """Per-chunk stat accumulation right after h[m] updates; also
                refreshes the persistent bf16 mirror hb[m]."""
                s1, s2 = s12
                nc.vector.tensor_copy(out=hb[m][:], in_=src_m[:])
                hsq = sb.tile([128, T2], BF16, tag="hsq", bufs=1)
                nc.vector.tensor_mul(out=hsq[:], in0=hb[m][:], in1=hb[m][:])
                nc.tensor.matmul(s1[:], ones_b[:], hb[m][:], start=(m == 0), stop=(m == ND - 1))
                nc.tensor.matmul(s2[:], ones_b[:], hsq[:], start=(m == 0), stop=(m == ND - 1))

            def ln_chain(s12):
                """Scalar chain: mean, -mean (f32r), rstd broadcast rb_s.
                rstd = exp(-0.5*ln(s2/D - mean^2 + eps))."""
                s1, s2 = s12
                ns = {}
                mean = sb.tile([1, T2], F32, tag="lnstat", bufs=7)
                nc.vector.tensor_scalar_mul(out=mean[:], in0=s1[:], scalar1=1.0 / D)
                nmean_r = sb.tile([1, T2], BF16, tag="lnstat", bufs=7)
                nc.vector.tensor_scalar_mul(out=nmean_r[:], in0=s1[:], scalar1=-1.0 / D)
                msq = sb.tile([1, T2], F32, tag="lnstat", bufs=7)
                nc.vector.tensor_mul(out=msq[:], in0=mean[:], in1=mean[:])
                veps = sb.tile([1, T2], F32, tag="lnstat", bufs=7)
                nc.vector.scalar_tensor_tensor(
                    out=veps[:], in0=s2[:], scalar=1.0 / D, in1=msq[:],
                    op0=OP.mult, op1=OP.subtract)
                lnv = sb.tile([1, T2], F32, tag="lnstat", bufs=7)
                nc.scalar.activation(lnv[:], veps[:], AF.Ln, bias=eps_t[:])
                rstd = sb.tile([1, T2], F32, tag="lnstat", bufs=7)
                nc.scalar.activation(rstd[:], lnv[:], AF.Exp, scale=-0.5)
                rstd_r = sb.tile([1, T2], F32R, tag="lnstat", bufs=7)
                nc.vector.tensor_copy(out=rstd_r[:], in_=rstd[:])
                ns["mean"] = mean
                ns["nmean_r"] = nmean_r
                ns["rstd"] = rstd
                ns["rstd_r"] = rstd_r
                return ns

            def ln_chain_bcast(ns):
                """rstd row -> full [128,T2] broadcast. Emitted separately so
                the PE-queue slot lands after enough independent matmuls to
                cover the scalar chain latency (the PE queue is in-order)."""
                rb = ps.tile([128, T2], F32, tag="sc", bufs=2)
                nc.tensor.matmul(rb[:], onesM_r[:], ns["rstd_r"][:], start=True, stop=True)
                rb_s = sb.tile([128, T2], F32, tag="rb_s", bufs=1)
                nc.vector.tensor_copy(out=rb_s[:], in_=rb[:])
                ns["rb_s"] = rb_s

            def ln_apply_start(ns):
                """Broadcast mean*rstd for the 2-DVE-op apply."""
                mr_r = sb.tile([1, T2], F32R, tag="lnstat", bufs=7)
                nc.vector.tensor_mul(out=mr_r[:], in0=ns["mean"][:], in1=ns["rstd"][:])
                mb = ps.tile([128, T2], F32, tag="sc", bufs=2)
                nc.tensor.matmul(mb[:], onesM_r[:], mr_r[:], start=True, stop=True)
                mb_s = sb.tile([128, T2], F32, tag="mb_s", bufs=1)
                nc.vector.tensor_copy(out=mb_s[:], in_=mb[:])
                ns["mb_s"] = mb_s

            def ln_apply_chunk(ns, src_m, a_m):
                t1 = sb.tile([128, T2], F32, tag="lnt", bufs=1)
                nc.vector.scalar_tensor_tensor(
                    out=t1[:], in0=src_m[:], scalar=1.0, in1=ns["rb_s"][:],
                    op0=OP.mult, op1=OP.mult)
                nc.vector.scalar_tensor_tensor(
                    out=a_m[:], in0=t1[:], scalar=1.0, in1=ns["mb_s"][:],
                    op0=OP.mult, op1=OP.subtract)

            def ln_apply(ns, src, tag="a"):
                a = [sb.tile([128, T2], BF16, tag=f"{tag}{m}", bufs=1,
                             name=f"a_{tag}_{next(_ln_ctr)}_{m}") for m in range(ND)]
                if "rb_s" not in ns:
                    ln_chain_bcast(ns)
                ln_apply_start(ns)
                for m in range(ND):
                    ln_apply_chunk(ns, src[m], a[m])
                return a

            # stats for LN1 of layer 0 (h0 just loaded)
            s12 = ln_stats_tile()
            for m in range(ND):
                ln_stats(s12, h[m], m)

            # ---------------- layers ----------------
            for l in range(nlayers):
                prm = sb.tile([128, NPRM], F32, tag="prm", bufs=2)
                nc.sync.dma_start(prm[:], prm_d[l])
                wsk_r = sb.tile([1, ND, 128], BF16, tag="wskr", bufs=1)
                nc.sync.dma_start(wsk_r[:], wsk_d[l])
                wsf_r = sb.tile([1, NF, 128], BF16, tag="wsfr", bufs=1)
                nc.sync.dma_start(wsf_r[:], wsf_d[l])

                ns1 = ln_chain(s12)

                k_in = dram.tile([KV_K], BF16, tag="k_in", bufs=2, name=f"k_in{l}")
                k_out = dram.tile([2 * KV_K], BF16, tag="k_out", bufs=2, name=f"k_out{l}")
                v_in = dram.tile([KV_V], BF16, tag="v_in", bufs=2, name=f"v_in{l}")
                v_out = dram.tile([2 * KV_V], BF16, tag="v_out", bufs=2, name=f"v_out{l}")
                kin_k = k_in[:].rearrange("(m ki t) -> ki m t", ki=128, t=T2)
                kin_v = v_in[:].rearrange("(p f) -> p f", f=780)

                # K projection straight off the raw bf16 residual mirror: the
                # mean term enters as a rank-1 accumulate (wsumK x -mean), the
                # rstd scale rides the PSUM evacuation; nothing waits for the
                # serial LN apply. The rank-1 + evacuation of chunk m are
                # emitted one group late (and the rstd broadcast after group 1)
                # so the in-order PE queue never parks on the scalar chain.
                kl = [sb.tile([128, T2], BF16, tag=f"kl{m}", bufs=1, name=f"kl{l}_{m}") for m in range(ND)]
                a1 = [sb.tile([128, T2], BF16, tag=f"a{m}", bufs=1,
                              name=f"a1_{l}_{m}") for m in range(ND)]
                pks = {}

                def k_finish(m):
                    nc.tensor.matmul(pks[m][:], wsk_r[0:1, m, :], ns1["nmean_r"][:], start=False, stop=True)
                    nc.vector.scalar_tensor_tensor(
                        out=kl[m][:], in0=pks[m][:], scalar=1.0, in1=ns1["rb_s"][:],
                        op0=OP.mult, op1=OP.mult)
                    nc.sync.dma_start(kin_k[:, m, :], kl[m][:])

                for m in range(ND):
                    wk_sl = wp.tile([128, ND, 128], BF16, tag="wk", bufs=2)
                    nc.sync.dma_start(wk_sl[:], wk_d[l, m])
                    pk = ps.tile([128, T2], F32, tag="mm", bufs=2, name=f"pk_{l}_{m}")
                    pks[m] = pk
                    for k in range(ND):
                        nc.tensor.matmul(pk[:], wk_sl[:, k], hb[k][:], start=(k == 0), stop=False)
                    if m == 1:
                        ln_chain_bcast(ns1)
                        ln_apply_start(ns1)
                    if m >= 1:
                        k_finish(m - 1)
                    if m >= 2:
                        ln_apply_chunk(ns1, h[m - 2], a1[m - 2])
                k_finish(ND - 1)
                nc.gpsimd.collective_compute(
                    "AllGather", OP.bypass,
                    ins=[k_in[:].opt()], outs=[k_out[:].opt()], replica_groups=rg)
                for m in range(ND - 2, ND):
                    ln_apply_chunk(ns1, h[m], a1[m])

                # V projection (token-major, 65-strided heads + ones col)
                vl = [sb.tile([128, 780], BF16, tag=f"vl{t}", bufs=1, name=f"vl{l}_{t}") for t in range(NT)]
                wv_sl = wp.tile([128, ND, D], BF16, tag="wv", bufs=1)
                nc.sync.dma_start(wv_sl[:], wv_d[l])
                for t in range(NT):
                    pv1 = ps.tile([128, T2], F32, tag="mm", bufs=2)
                    pv2 = ps.tile([128, 256], F32, tag="mm", bufs=2)
                    for k in range(ND):
                        lhs = a1[k][:, 128 * t : 128 * t + 128]
                        nc.tensor.matmul(pv1[:], lhs, wv_sl[:, k, 0:512], start=(k == 0), stop=(k == ND - 1))
                        nc.tensor.matmul(pv2[:], lhs, wv_sl[:, k, 512:768], start=(k == 0), stop=(k == ND - 1))
                    vch = vl[t][:].rearrange("p (h e) -> p h e", e=65)
                    nc.vector.tensor_copy(
                        out=vch[:, 0:8, 0:64],
                        in_=pv1[:].rearrange("p (h e) -> p h e", e=64))
                    nc.vector.tensor_copy(
                        out=vch[:, 8:12, 0:64],
                        in_=pv2[:].rearrange("p (h e) -> p h e", e=64))
                    nc.vector.memset(vch[:, :, 64:65], 1.0)
                    nc.sync.dma_start(kin_v[128 * t : 128 * t + 128, :], vl[t][:])

                nc.gpsimd.collective_compute(
                    "AllGather", OP.bypass,
                    ins=[v_in[:].opt()], outs=[v_out[:].opt()], replica_groups=rg)

                # Q projection (feature-major, stays local)
                q = [sb.tile([128, T2], BF16, tag=f"q{m}", bufs=1, name=f"q{l}_{m}") for m in range(ND)]
                for m in range(ND):
                    wq_sl = wp.tile([128, ND, 128], BF16, tag="wq", bufs=2)
                    nc.sync.dma_start(wq_sl[:], wq_d[l, m])
                    pq = ps.tile([128, T2], F32, tag="mm", bufs=2)
                    for k in range(ND):
                        nc.tensor.matmul(pq[:], wq_sl[:, k], a1[k][:], start=(k == 0), stop=(k == ND - 1))
                    nc.vector.tensor_copy(out=q[m][:], in_=pq[:])

                # gathered rank-0 half: K (feature-major) / V-hat (token-major)
                kg = sb.tile([128, ND, T2], BF16, tag="kg", bufs=1)
                nc.sync.dma_start(
                    kg[:], k_out[0:KV_K].rearrange("(m ki t) -> ki m t", ki=128, t=T2))
                vg = sb.tile([128, NT, 780], BF16, tag="vg", bufs=1)
                nc.sync.dma_start(
                    vg[:], v_out[0:KV_V].rearrange("(to ti f) -> ti to f", ti=128, f=780))

                # ---- attention ----
                # phase A: own keys from SBUF (kl/vl), causal-trimmed, triangle
                # mask on the diagonal 128-query block only.
                # phase B: gathered rank-0 keys, full 512 queries; contribution
                # nulled on seq-half-0 cores via additive exp bias.
                o = [sb.tile([128, T2], BF16, tag=f"o{m}", bufs=1, name=f"o{l}_{m}") for m in range(ND)]
                poA_s = [sb.tile([65, 2, T2], BF16, tag=f"poa{m}", bufs=1, name=f"poa{l}_{m}") for m in range(ND)]
                dng = [sb.tile([6, T2], F32, tag=f"dn{g}", bufs=1, name=f"dn{l}_{g}") for g in range(2)]

                for hp in range(ND):
                    poA = ps.tile([65, 2, T2], F32, tag="po", bufs=1)
                    for tk in range(NT):
                        qlo = 128 * tk
                        s2b = ps.tile([128, 2, T2], F32, tag="sc", bufs=2)
                        for j in range(2):
                            nc.tensor.matmul(
                                s2b[:, j, qlo:],
                                kl[hp][64 * j : 64 * j + 64, qlo : qlo + 128],
                                q[hp][64 * j : 64 * j + 64, qlo:],
                                start=True, stop=True)
                        p2 = sb.tile([128, 2, T2], BF16, tag="p", bufs=2)
                        nc.scalar.activation(p2[:, :, qlo:], s2b[:, :, qlo:], AF.Exp, scale=SCALE)
                        for j in range(2):
                            nc.vector.tensor_mul(
                                out=p2[:, j, qlo : qlo + 128],
                                in0=p2[:, j, qlo : qlo + 128], in1=tri[:])
                            nc.tensor.matmul(
                                poA[:, j, qlo:],
                                vl[tk][:, 65 * (2 * hp + j) : 65 * (2 * hp + j) + 65],
                                p2[:, j, qlo:],
                                start=(tk == 0), stop=(tk == NT - 1))
                    nc.vector.tensor_copy(out=poA_s[hp][:], in_=poA[:])

                for hp in (3, 4, 5, 0, 1, 2):
                    poB = ps.tile([65, 2, T2], F32, tag="po", bufs=1)
                    for tr in range(NT):
                        s2b = ps.tile([128, 2, T2], F32, tag="sc", bufs=2)
                        for j in range(2):
                            nc.tensor.matmul(
                                s2b[:, j, :],
                                kg[64 * j : 64 * j + 64, hp, 128 * tr : 128 * tr + 128],
                                q[hp][64 * j : 64 * j + 64, :],
                                start=True, stop=True)
                        p2 = sb.tile([128, 2, T2], BF16, tag="p", bufs=2)
                        nc.scalar.activation(p2[:], s2b[:], AF.Exp, scale=SCALE, bias=bB[:, 0:1])
                        for j in range(2):
                            nc.tensor.matmul(
                                poB[:, j, :],
                                vg[:, tr, 65 * (2 * hp + j) : 65 * (2 * hp + j) + 65],
                                p2[:, j, :],
                                start=(tr == 0), stop=(tr == NT - 1))
                    # combine phases; split the ones-row into the denominator tile
                    g = hp // 3
                    for j in range(2):
                        hi = 2 * hp + j
                        nc.vector.tensor_tensor(
                            out=o[hp][64 * j : 64 * j + 64, :],
                            in0=poB[0:64, j, :], in1=poA_s[hp][0:64, j, :], op=OP.add)
                        dtmp = sb.tile([1, T2], F32, tag="dtmp", bufs=2)
                        nc.vector.tensor_tensor(
                            out=dtmp[:], in0=poB[64:65, j, :], in1=poA_s[hp][64:65, j, :], op=OP.add)
                        nc.sync.dma_start(dng[g][(hi - 6 * g) : (hi - 6 * g) + 1, :], dtmp[:])
                    if hp in (5, 2):
                        # normalize group ng as soon as its three head-pairs
                        # finished (g1's chunks run first so the trailing g0
                        # chain overlaps the O-projection's k=3..5 half):
                        # 1/denom = exp(-ln(denom)); head broadcast via selector
                        ng = 1 if hp == 5 else 0
                        sel = selA_r if ng == 0 else selB_r
                        nc.scalar.activation(dng[ng][:], dng[ng][:], AF.Ln)
                        rec_r = sb.tile([6, T2], BF16, tag=f"recr{ng}", bufs=1, name=f"recr{l}_{ng}")
                        nc.scalar.activation(rec_r[:], dng[ng][:], AF.Exp, scale=-1.0)
                        for m in range(3 * ng, 3 * ng + 3):
                            dnb = ps.tile([128, T2], F32, tag="mm", bufs=2)
                            nc.tensor.matmul(dnb[:], sel[:, m, :], rec_r[:], start=True, stop=True)
                            nc.vector.scalar_tensor_tensor(
                                out=o[m][:], in0=o[m][:], scalar=1.0,
                                in1=dnb[:], op0=OP.mult, op1=OP.mult)

                # output projection + residual; LN2 stats chase the h updates.
                # k runs g1-heads (3,4,5) first: the m=0/1 first halves execute
                # while group 0's denominator chain finishes.
                s12 = ln_stats_tile()
                KORD = (3, 4, 5, 0, 1, 2)
                pws = {}
                wos = {}
                for m in (0, 1):
                    wos[m] = wp.tile([128, ND, 128], BF16, tag="wo", bufs=2, name=f"wo_{l}_{m}")
                    nc.sync.dma_start(wos[m][:], wo_d[l, m])
                    pws[m] = ps.tile([128, T2], F32, tag="mm", bufs=2, name=f"pw_{l}_{m}")
                    for k in (3, 4, 5):
                        nc.tensor.matmul(pws[m][:], wos[m][:, k], o[k][:], start=(k == 3), stop=False)

                def o_finish(m, pw):
                    tt = sb.tile([128, T2], BF16, tag="res", bufs=2)
                    nc.scalar.activation(tt[:], pw[:], AF.Identity, bias=prm[:, P_BO + m : P_BO + m + 1])
                    nc.vector.tensor_tensor(out=h[m][:], in0=h[m][:], in1=tt[:], op=OP.add)
                    ln_stats(s12, h[m], m)

                for m in (0, 1):
                    for k in (0, 1, 2):
                        nc.tensor.matmul(pws[m][:], wos[m][:, k], o[k][:], start=False, stop=(k == 2))
                    o_finish(m, pws[m])
                for m in range(2, ND):
                    wo_sl = wp.tile([128, ND, 128], BF16, tag="wo", bufs=2)
                    nc.sync.dma_start(wo_sl[:], wo_d[l, m])
                    pw = ps.tile([128, T2], F32, tag="mm", bufs=2)
                    for k in KORD:
                        nc.tensor.matmul(pw[:], wo_sl[:, k], o[k][:], start=(k == 3), stop=(k == 2))
                    o_finish(m, pw)

                # FFN, raw-path: w1 projects the raw mirror with a rank-1 mean
                # correction; relu commutes with the (positive) per-token rstd,
                # which is deferred to the w2 evacuation (needs b1 == b2 == 0).
                ns2 = ln_chain(s12)
                f = [sb.tile([128, T2], BF16, tag=f"f{fc}", bufs=1, name=f"f{l}_{fc}") for fc in range(NF)]
                pfs = {}

                def w1_finish(fc):
                    nc.tensor.matmul(pfs[fc][:], wsf_r[0:1, fc, :], ns2["nmean_r"][:], start=False, stop=True)
                    nc.scalar.activation(f[fc][:], pfs[fc][:], AF.Relu)

                for fc in range(NF):
                    w1_sl = wp.tile([128, ND, 128], BF16, tag="w1", bufs=2)
                    nc.sync.dma_start(w1_sl[:], w1_d[l, fc])
                    pf = ps.tile([128, T2], F32, tag=("mm" if fc % 2 == 0 else "sc"), bufs=2, name=f"pf_{l}_{fc}")
                    pfs[fc] = pf
                    for k in range(ND):
                        nc.tensor.matmul(pf[:], w1_sl[:, k], hb[k][:], start=(k == 0), stop=False)
                    if fc == 1:
                        ln_chain_bcast(ns2)
                    if fc >= 1:
                        w1_finish(fc - 1)
                w1_finish(NF - 1)
                s12 = ln_stats_tile()
                for m in range(ND):
                    w2_sl = wp.tile([128, NF, 128], BF16, tag="w2", bufs=2)
                    nc.sync.dma_start(w2_sl[:], w2_d[l, m])
                    pg = ps.tile([128, T2], F32, tag="mm", bufs=2)
                    for k in range(NF):
                        nc.tensor.matmul(pg[:], w2_sl[:, k], f[k][:], start=(k == 0), stop=(k == NF - 1))
                    tt = sb.tile([128, T2], BF16, tag="res", bufs=2)
                    nc.vector.scalar_tensor_tensor(
                        out=tt[:], in0=pg[:], scalar=1.0, in1=ns2["rb_s"][:],
                        op0=OP.mult, op1=OP.mult)
                    nc.vector.tensor_tensor(out=h[m][:], in0=h[m][:], in1=tt[:], op=OP.add)
                    ln_stats(s12, h[m], m)

            # ---------------- final LN + head ----------------
            ns_f = ln_chain(s12)
            hf_t = ln_apply(ns_f, h, tag="a")
            for v in range(NV):
                wh_sl = wp.tile([128, ND, 512], BF16, tag="wh", bufs=3)
                nc.sync.dma_start(wh_sl[:], wh_d[v])
                o_dst = out_d[:, 512 * v : 512 * v + 512].rearrange("(to ti) f -> ti to f", ti=128)
                for t in range(NT):
                    pl = ps.tile([128, 512], F32, tag="mm", bufs=2)
                    for k in range(ND):
                        nc.tensor.matmul(
                            pl[:], hf_t[k][:, 128 * t : 128 * t + 128], wh_sl[:, k],
                            start=(k == 0), stop=(k == ND - 1))
                    lg = sb.tile([128, 512], BF16, tag="lg", bufs=4)
                    # evacuate on ScalarE: the DVE is the busier engine here
                    nc.scalar.activation(lg[:], pl[:], AF.Identity)
                    nc.sync.dma_start(o_dst[:, t], lg[:])

    nc.compile()
    if not nc.is_finalized():
        nc.finalize()
    return nc


def _prep_shared(inputs, nlayers):
    bf = ml_dtypes.bfloat16
    for k in ("ln1_b", "ln2_b", "lnf_b", "b1", "b2"):
        assert not np.any(np.asarray(inputs[k])), f"{k} must be zero (folded LN/FFN)"
    wq, wk, wv, wo = (np.asarray(inputs[k], np.float32) for k in ("wq", "wk", "wv", "wo"))
    w1, w2 = np.asarray(inputs["w1"], np.float32), np.asarray(inputs["w2"], np.float32)
    w_head = np.asarray(inputs["w_head"], np.float32)
    ln1_s = np.asarray(inputs["ln1_s"], np.float32)[:nlayers]
    ln2_s = np.asarray(inputs["ln2_s"], np.float32)[:nlayers]
    lnf_s = np.asarray(inputs["lnf_s"], np.float32)

    # fold LN scales into the consuming projections
    wq = wq[:nlayers] * ln1_s[:, :, None]
    wk = wk[:nlayers] * ln1_s[:, :, None]
    wv = wv[:nlayers] * ln1_s[:, :, None]
    w1 = w1[:nlayers] * ln2_s[:, :, None]
    w_head = w_head * lnf_s[:, None]

    def lhst(w, nm, nk):
        # [L, nk*128, nm*128] -> [L, nm, 128, nk, 128] with [l,m,ki,ko,j] = w[l,128ko+ki,128m+j]
        return np.ascontiguousarray(
            w[:nlayers].reshape(nlayers, nk, 128, nm, 128).transpose(0, 3, 2, 1, 4)).astype(bf)

    d = {}
    d["wq_t"] = lhst(wq, ND, ND)
    d["wk_t"] = lhst(wk, ND, ND)
    d["wo_t"] = lhst(wo, ND, ND)
    d["w1_t"] = lhst(w1, NF, ND)
    d["w2_t"] = lhst(w2, ND, NF)
    d["wv_t"] = np.ascontiguousarray(
        wv.reshape(nlayers, ND, 128, D).transpose(0, 2, 1, 3)).astype(bf)
    # column sums of the (scale-folded) K / w1 weights, for the rank-1 mean
    # corrections; summed in fp32 AFTER the bf16 rounding the device will see.
    d["wsumK"] = np.ascontiguousarray(
        wk.astype(bf).astype(np.float32).sum(axis=1).reshape(nlayers, 1, ND, 128)).astype(bf)
    d["wsumF"] = np.ascontiguousarray(
        w1.astype(bf).astype(np.float32).sum(axis=1).reshape(nlayers, 1, NF, 128)).astype(bf)
    whp = np.concatenate([w_head, np.zeros((D, VP - V), np.float32)], axis=1)
    d["wh_t"] = np.ascontiguousarray(
        whp.reshape(ND, 128, NV, 512).transpose(2, 1, 0, 3)).astype(bf)

    prm = np.zeros((nlayers, 128, NPRM), np.float32)

    def chunked(a):  # [L, 768] -> [L, 128, n]
        return np.asarray(a, np.float32)[:nlayers].reshape(nlayers, -1, 128).transpose(0, 2, 1)

    prm[:, :, P_BO : P_BO + ND] = chunked(inputs["bo"])
    prm[:, :, P_B2 : P_B2 + ND] = chunked(inputs["b2"])
    prm[:, :, P_B1 : P_B1 + NF] = chunked(inputs["b1"])
    d["prm"] = np.ascontiguousarray(prm)

    kk = np.arange(128)[:, None]
    qq = np.arange(128)[None, :]
    d["tri"] = np.ascontiguousarray((kk <= qq).astype(bf))

    selA = np.zeros((6, ND, 128), np.float32)
    selB = np.zeros((6, ND, 128), np.float32)
    for hi in range(H):
        tgt = selA if hi < 6 else selB
        tgt[hi % 6, hi // 2, 64 * (hi % 2) : 64 * (hi % 2) + 64] = 1.0
    d["selA"] = selA.astype(bf)
    d["selB"] = selB.astype(bf)
    return d


_LAST_RESULTS = None


def kernel(x, tok_emb, pos_emb, wq, wk, wv, wo, bo, ln1_s, ln1_b,
           ln2_s, ln2_b, w1, b1, w2, b2, lnf_s, lnf_b, w_head, b_head,
           nlayers=L):
    global _LAST_RESULTS
    if nlayers not in _BUILT:
        _BUILT[nlayers] = _build(nlayers)
    nc = _BUILT[nlayers]

    inputs = dict(x=x, tok_emb=tok_emb, pos_emb=pos_emb, wq=wq, wk=wk, wv=wv,
                  wo=wo, bo=bo, ln1_s=ln1_s, ln1_b=ln1_b, ln2_s=ln2_s,
                  ln2_b=ln2_b, w1=w1, b1=b1, w2=w2, b2=b2, lnf_s=lnf_s,
                  lnf_b=lnf_b, w_head=w_head, b_head=b_head)
    shared = _prep_shared(inputs, nlayers)

    xi = np.asarray(x).astype(np.int64)
    te = np.asarray(tok_emb, np.float32)
    pe = np.asarray(pos_emb, np.float32)[:T]
    h0 = te[xi] + pe[None, :, :]  # [B, T, D] fp32

    in_maps = []
    for c in range(8):
        b, s = c // 2, c % 2
        hc = np.ascontiguousarray(
            h0[b, T2 * s : T2 * (s + 1)].T.reshape(ND, 128, T2).transpose(1, 0, 2))
        m = {"h0": hc,
             "biasB": np.full((128, 1), 0.0 if s == 1 else -1e5, np.float32)}
        m.update(shared)
        in_maps.append(m)

    res = bass_utils.run_bass_kernel_spmd(nc, in_maps, core_ids=list(range(8)))
    _LAST_RESULTS = res

    out = np.empty((B, T, V), np.float32)
    for c in range(8):
        b, s = c // 2, c % 2
        out[b, T2 * s : T2 * (s + 1)] = res.results[c]["logits"][:, :V].astype(np.float32)
    bh = np.asarray(b_head, np.float32)
    if np.any(bh):
        out += bh
    return out


if __name__ == "__main__":
    nl = int(os.environ.get("KERNEL_LAYERS", L))
    _build(nl)
    print("build ok", nl)
